# revision 1
# baseline (speedup 1.0000x reference)
import sys

sys.path.insert(0, "/opt/trn_rl_repo")

import ml_dtypes
import numpy as np

import concourse.bass as bass
import concourse.tile as tile
from concourse import bacc, mybir
from concourse.bass_utils import run_bass_kernel_spmd

F32 = mybir.dt.float32
F32R = mybir.dt.float32r
BF16 = mybir.dt.bfloat16
AF = mybir.ActivationFunctionType

BATCH = 2
SEQ = 2048
D = 1024
NHEADS = 16
DK = 64
HPC = 4          # heads per core
NCORES = 8
THETA = 10000.0
EPS = 1e-8
NEG = -30000.0
CHUNK = 512
NCH = SEQ // CHUNK   # 4 chunks of queries
NBLK = SEQ // 128    # 16 key blocks


def _build_nc():
    nc = bacc.Bacc("TRN2", target_bir_lowering=False)
    XT = nc.declare_dram_parameter("XT", [128, 8, SEQ], BF16, isOutput=False)
    WT = nc.declare_dram_parameter("WT", [128, 8, 768], BF16, isOutput=False)
    COS = nc.declare_dram_parameter("COS", [128, SEQ], BF16, isOutput=False)
    SIN = nc.declare_dram_parameter("SIN", [128, SEQ], BF16, isOutput=False)
    WOT = nc.declare_dram_parameter("WOT", [128, 2, D], BF16, isOutput=False)
    MASKB = nc.declare_dram_parameter("MASKB", [128, 896], BF16, isOutput=False)
    INDT = nc.declare_dram_parameter("INDT", [128, 4], BF16, isOutput=False)
    I2Q = nc.declare_dram_parameter("I2Q", [4, 128], F32, isOutput=False)
    I2K = nc.declare_dram_parameter("I2K", [4, 128], F32, isOutput=False)
    ID = nc.declare_dram_parameter("ID", [128, 128], BF16, isOutput=False)
    OUT = nc.declare_dram_parameter("OUT", [SEQ, D], F32, isOutput=True)

    with tile.TileContext(nc) as tc:
        with (
            nc.allow_low_precision(reason="bf16 matmuls validated at 1e-2 rel err"),
            tc.tile_pool(name="cst", bufs=1) as cst,
            tc.tile_pool(name="xtp", bufs=2) as xtp,
            tc.tile_pool(name="tmp", bufs=8) as tmp,
            tc.tile_pool(name="expp", bufs=3) as expp,
            tc.tile_pool(name="bcp", bufs=2) as bcp,
            tc.tile_pool(name="ocp", bufs=2) as ocp,
            tc.tile_pool(name="ps", bufs=2, space="PSUM") as ps,
        ):
            wt_sb = cst.tile([128, 8, 768], BF16, tag="wt")
            cos_sb = cst.tile([128, SEQ], BF16, tag="cos")
            sin_sb = cst.tile([128, SEQ], BF16, tag="sin")
            wot_sb = cst.tile([128, 2, D], BF16, tag="wot")
            mask_sb = cst.tile([128, 896], BF16, tag="mask")
            indt_sb = cst.tile([128, 4], BF16, tag="indt")
            i2q_sb = cst.tile([4, 128], F32, tag="i2q")
            i2k_sb = cst.tile([4, 128], F32, tag="i2k")
            id_sb = cst.tile([128, 128], BF16, tag="id")
            q_sb = cst.tile([128, 2, SEQ], BF16, tag="q")
            k_sb = cst.tile([128, 2, SEQ], BF16, tag="k")
            v_sb = cst.tile([128, NBLK, HPC, 65], BF16, tag="v")
            ot_sb = cst.tile([128, 2, SEQ], BF16, tag="ot")
            ones1 = cst.tile([1, 64], F32, tag="ones1")

            def emit_xt(c):
                c0 = c * CHUNK
                xt_t = xtp.tile([128, 8, CHUNK], BF16, tag="xt", name=f"xt_{c}")
                nc.sync.dma_start(out=xt_t[:], in_=XT[:, :, c0:c0 + CHUNK])
                return xt_t

            # startup: first proj needs xt(0) + QK weight columns; issue those
            # DMAs ahead of everything else
            xt0 = emit_xt(0)
            nc.sync.dma_start(out=wt_sb[:, :, 0:512], in_=WT[:, :, 0:512])
            nc.sync.dma_start(out=indt_sb[:], in_=INDT[:])
            nc.sync.dma_start(out=i2q_sb[:].bitcast(F32R), in_=I2Q[:].bitcast(F32R))
            nc.sync.dma_start(out=i2k_sb[:].bitcast(F32R), in_=I2K[:].bitcast(F32R))
            nc.sync.dma_start(out=cos_sb[:], in_=COS[:])
            nc.sync.dma_start(out=sin_sb[:], in_=SIN[:])

            # ones columns for the denominator trick (data cols overwritten below)
            nc.vector.memset(v_sb[:], 1.0)
            nc.vector.memset(ones1[:], 1.0)
            nc.vector.tensor_copy(ones1[:].bitcast(F32R), ones1[:])

            def emit_proj_qk(c, qk, xt_t):
                c0 = c * CHUNK
                qoff = 256 * qk
                dst = q_sb if qk == 0 else k_sb
                i2 = i2q_sb if qk == 0 else i2k_sb

                pA = ps.tile([128, CHUNK], F32, tag="pp", name=f"pA_{qk}_{c}")
                for di in range(8):
                    nc.tensor.matmul(
                        pA,
                        lhsT=wt_sb[:, di, qoff:qoff + 128],
                        rhs=xt_t[:, di, :],
                        start=(di == 0), stop=(di == 7),
                    )
                pB = ps.tile([128, CHUNK], F32, tag="pp", name=f"pB_{qk}_{c}")
                for di in range(8):
                    nc.tensor.matmul(
                        pB,
                        lhsT=wt_sb[:, di, qoff + 128:qoff + 256],
                        rhs=xt_t[:, di, :],
                        start=(di == 0), stop=(di == 7),
                    )

                sqA = tmp.tile([128, CHUNK], BF16, tag="t", name=f"sqA_{qk}_{c}")
                nc.scalar.activation(sqA[:], pA[:], AF.Square)
                sqB = tmp.tile([128, CHUNK], BF16, tag="t", name=f"sqB_{qk}_{c}")
                nc.scalar.activation(sqB[:], pB[:], AF.Square)
                ssum = tmp.tile([128, CHUNK], BF16, tag="t", name=f"ssum_{qk}_{c}")
                nc.vector.tensor_add(ssum[:], sqA[:], sqB[:])

                n2 = ps.tile([4, CHUNK], F32, tag="mm", name=f"n2_{qk}_{c}")
                nc.tensor.matmul(n2, lhsT=indt_sb[:], rhs=ssum[:],
                                 start=True, stop=True)
                nrm = tmp.tile([4, CHUNK], F32, tag="t", name=f"nrm_{qk}_{c}")
                nc.scalar.activation(nrm[:], n2[:], AF.Sqrt)
                nc.vector.tensor_scalar_add(nrm[:], nrm[:], EPS)
                nrmr = tmp.tile([4, CHUNK], F32, tag="t", name=f"nrmr_{qk}_{c}")
                nc.vector.reciprocal(nrmr[:].bitcast(F32R), nrm[:])

                rbp = ps.tile([128, CHUNK], F32, tag="mm", name=f"rbp_{qk}_{c}")
                nc.tensor.matmul(
                    rbp, lhsT=i2[:].bitcast(F32R),
                    rhs=nrmr[:].bitcast(F32R), start=True, stop=True,
                )
                rb = tmp.tile([128, CHUNK], BF16, tag="t", name=f"rb_{qk}_{c}")
                nc.vector.tensor_copy(rb[:], rbp[:])

                cs = cos_sb[:, c0:c0 + CHUNK]
                sn = sin_sb[:, c0:c0 + CHUNK]
                rbc = tmp.tile([128, CHUNK], BF16, tag="t", name=f"rbc_{qk}_{c}")
                nc.vector.tensor_mul(rbc[:], rb[:], cs)
                rbs = tmp.tile([128, CHUNK], BF16, tag="t", name=f"rbs_{qk}_{c}")
                nc.vector.tensor_mul(rbs[:], rb[:], sn)

                tac = tmp.tile([128, CHUNK], BF16, tag="t", name=f"tac_{qk}_{c}")
                nc.vector.tensor_mul(tac[:], pA[:], rbc[:])
                tas = tmp.tile([128, CHUNK], BF16, tag="t", name=f"tas_{qk}_{c}")
                nc.vector.tensor_mul(tas[:], pA[:], rbs[:])
                tbc = tmp.tile([128, CHUNK], BF16, tag="t", name=f"tbc_{qk}_{c}")
                nc.vector.tensor_mul(tbc[:], pB[:], rbc[:])
                tbs = tmp.tile([128, CHUNK], BF16, tag="t", name=f"tbs_{qk}_{c}")
                nc.vector.tensor_mul(tbs[:], pB[:], rbs[:])

                for h in range(HPC):
                    po = (h % 2) * 64
                    ti = h // 2
                    hs = 32 * h
                    nc.vector.tensor_sub(
                        dst[po:po + 32, ti, c0:c0 + CHUNK],
                        tac[hs:hs + 32, :], tbs[hs:hs + 32, :])
                    nc.vector.tensor_add(
                        dst[po + 32:po + 64, ti, c0:c0 + CHUNK],
                        tas[hs:hs + 32, :], tbc[hs:hs + 32, :])

            def emit_proj_v(c, half, xt_t):
                for bb in (2 * half, 2 * half + 1):
                    nb = 4 * c + bb
                    vps = ps.tile([128, HPC, 64], F32, tag="mm", name=f"vps_{nb}")
                    for di in range(8):
                        nc.tensor.matmul(
                            vps,
                            lhsT=xt_t[:, di, bb * 128:bb * 128 + 128],
                            rhs=wt_sb[:, di, 512:768],
                            start=(di == 0), stop=(di == 7),
                        )
                    nc.vector.tensor_copy(v_sb[:, nb, :, 0:64], vps[:])

            def emit_attn_head(c, h):
                c0 = c * CHUNK
                njb = 4 * (c + 1)
                po = (h % 2) * 64
                ti = h // 2
                av = ps.tile([65, CHUNK], F32, tag="av", name=f"av_{h}_{c}")

                def issue_sc(jb):
                    diag = jb >= 4 * c
                    sc = ps.tile([128, CHUNK], F32, tag="sc", name=f"sc_{h}_{c}_{jb}")
                    nc.tensor.matmul(
                        sc,
                        lhsT=k_sb[po:po + 64, ti, jb * 128:jb * 128 + 128],
                        rhs=q_sb[po:po + 64, ti, c0:c0 + CHUNK],
                        start=True, stop=not diag,
                    )
                    if diag:
                        s0 = 384 - 128 * (jb - 4 * c)
                        nc.tensor.matmul(
                            sc, lhsT=id_sb[:], rhs=mask_sb[:, s0:s0 + CHUNK],
                            start=False, stop=True,
                        )
                    return sc

                # stagger: issue sc for jb+1 before draining jb so PE
                # keeps ahead of ACT's exp stream
                cur = issue_sc(0)
                for jb in range(njb):
                    nxt = issue_sc(jb + 1) if jb + 1 < njb else None
                    ex = expp.tile([128, CHUNK], BF16, tag="ex",
                                   name=f"ex_{h}_{c}_{jb}")
                    nc.scalar.activation(ex[:], cur[:], AF.Exp)
                    nc.tensor.matmul(
                        av, lhsT=v_sb[:, jb, h, :], rhs=ex[:],
                        start=(jb == 0), stop=(jb == njb - 1),
                    )
                    cur = nxt

                srec = bcp.tile([1, CHUNK], F32, tag="srec", name=f"srec_{h}_{c}")
                nc.vector.reciprocal(srec[:].bitcast(F32R), av[64:65, :])
                rb2p = ps.tile([64, CHUNK], F32, tag="mm", name=f"rb2p_{h}_{c}")
                nc.tensor.matmul(
                    rb2p, lhsT=ones1[:].bitcast(F32R),
                    rhs=srec[:].bitcast(F32R), start=True, stop=True,
                )
                rb2 = bcp.tile([64, CHUNK], F32, tag="rb2", name=f"rb2_{h}_{c}")
                nc.vector.tensor_copy(rb2[:], rb2p[:])
                nc.vector.tensor_mul(
                    ot_sb[po:po + 64, ti, c0:c0 + CHUNK],
                    av[0:64, :], rb2[:])

            def emit_outproj(c):
                for bb in range(4):
                    nb = 4 * c + bb
                    for oc in range(2):
                        wo = ps.tile([128, CHUNK], F32, tag="mm", name=f"wo_{nb}_{oc}")
                        for ti in range(2):
                            nc.tensor.matmul(
                                wo,
                                lhsT=ot_sb[:, ti, nb * 128:nb * 128 + 128],
                                rhs=wot_sb[:, ti, oc * CHUNK:oc * CHUNK + CHUNK],
                                start=(ti == 0), stop=(ti == 1),
                            )
                        ob = ocp.tile([128, CHUNK], F32, tag="ob", name=f"ob_{nb}_{oc}")
                        nc.vector.tensor_copy(ob[:], wo[:])
                        nc.sync.dma_start(
                            out=OUT[nb * 128:nb * 128 + 128, oc * CHUNK:oc * CHUNK + CHUNK],
                            in_=ob[:])

            # warm-up row: chunk 0 projections, with remaining const DMAs
            # slotted where they are first needed
            emit_proj_qk(0, 0, xt0)
            nc.sync.dma_start(out=wt_sb[:, :, 512:768], in_=WT[:, :, 512:768])
            emit_proj_qk(0, 1, xt0)
            emit_proj_v(0, 0, xt0)
            nc.sync.dma_start(out=id_sb[:], in_=ID[:])
            nc.sync.dma_start(out=mask_sb[:], in_=MASKB[:])
            emit_proj_v(0, 1, xt0)
            nc.sync.dma_start(out=wot_sb[:], in_=WOT[:])

            # steady rows: interleave chunk c's projections (PE/DVE-heavy)
            # with chunk c-1's attention heads (ACT-heavy) so no engine
            # queue starves and the PE p-state stays ramped
            for c in range(1, NCH):
                xtc = emit_xt(c)
                emit_proj_qk(c, 0, xtc)
                emit_attn_head(c - 1, 0)
                emit_proj_qk(c, 1, xtc)
                emit_attn_head(c - 1, 1)
                emit_proj_v(c, 0, xtc)
                emit_attn_head(c - 1, 2)
                emit_proj_v(c, 1, xtc)
                emit_attn_head(c - 1, 3)
                emit_outproj(c - 1)

            for h in range(HPC):
                emit_attn_head(NCH - 1, h)
            emit_outproj(NCH - 1)
    return nc


_NC = None


def _get_nc():
    global _NC
    if _NC is None:
        _NC = _build_nc()
        _NC.finalize()
    return _NC


def _shared_tables(token_positions):
    freqs = np.arange(0, DK, 2, dtype=np.float64)
    inv_theta = THETA ** (-freqs / DK)                      # [32]
    pos = token_positions.astype(np.float64)
    ang = inv_theta[:, None] * pos[None, :]                 # [32, SEQ]
    cos_t = np.ascontiguousarray(
        np.tile(np.cos(ang), (4, 1))).astype(ml_dtypes.bfloat16)
    sin_t = np.ascontiguousarray(
        np.tile(np.sin(ang), (4, 1))).astype(ml_dtypes.bfloat16)

    p_i = np.arange(128)[:, None]
    t_i = np.arange(896)[None, :]
    maskb = np.where(t_i >= p_i + 384, 0.0, NEG).astype(ml_dtypes.bfloat16)

    indt = np.zeros((128, 4), dtype=np.float32)
    for j in range(4):
        indt[32 * j:32 * j + 32, j] = 1.0
    i2k = np.ascontiguousarray(indt.T)
    idm = np.eye(128, dtype=ml_dtypes.bfloat16)
    return cos_t, sin_t, maskb, indt.astype(ml_dtypes.bfloat16), i2k, idm


def _core_inputs(c, x, W_QKV, W_O, qk_scale, shared):
    cos_t, sin_t, maskb, indt, i2k, idm = shared
    b = c // 4
    a = c % 4
    heads = [4 * a + i for i in range(HPC)]

    qA = [64 * h + 2 * t for h in heads for t in range(32)]
    qB = [64 * h + 2 * t + 1 for h in heads for t in range(32)]
    kA = [1024 + r for r in qA]
    kB = [1024 + r for r in qB]
    vr = [2048 + 64 * h + j for h in heads for j in range(DK)]
    rows = qA + qB + kA + kB + vr
    wt = np.ascontiguousarray(
        W_QKV[rows, :].T.reshape(8, 128, 768).transpose(1, 0, 2)
    ).astype(ml_dtypes.bfloat16)

    vcols = [64 * h + j for h in heads for j in range(DK)]
    wot = np.ascontiguousarray(
        W_O[:, vcols].T.reshape(2, 128, D).transpose(1, 0, 2)
    ).astype(ml_dtypes.bfloat16)

    xt = np.ascontiguousarray(
        x[b].T.reshape(8, 128, SEQ).transpose(1, 0, 2)
    ).astype(ml_dtypes.bfloat16)

    i2q = np.zeros((4, 128), dtype=np.float32)
    for j in range(4):
        i2q[j, 32 * j:32 * j + 32] = np.float32(qk_scale[heads[j]])

    return {
        "XT": xt, "WT": wt, "COS": cos_t, "SIN": sin_t, "WOT": wot,
        "MASKB": maskb, "INDT": indt, "I2Q": i2q, "I2K": i2k, "ID": idm,
    }


def _run(inputs, trace=False):
    x = np.asarray(inputs["x"], dtype=np.float32)
    token_positions = np.asarray(inputs["token_positions"])
    W_QKV = np.asarray(inputs["W_QKV"], dtype=np.float32)
    W_O = np.asarray(inputs["W_O"], dtype=np.float32)
    qk_scale = np.asarray(inputs["qk_scale"], dtype=np.float32)

    shared = _shared_tables(token_positions)
    nc = _get_nc()
    in_maps = [_core_inputs(c, x, W_QKV, W_O, qk_scale, shared)
               for c in range(NCORES)]
    core_ids = list(range(NCORES))
    kw = {}
    if trace:
        kw = dict(trace=True, trace_cores=core_ids)
    res = run_bass_kernel_spmd(nc, in_maps, core_ids, **kw)
    parts = [np.asarray(r["OUT"], dtype=np.float32) for r in res.results]
    out = np.stack([
        parts[0] + parts[1] + parts[2] + parts[3],
        parts[4] + parts[5] + parts[6] + parts[7],
    ]).astype(np.float32)
    return out, getattr(res, "exec_time_ns", None)


def kernel(**inputs):
    return _run(inputs, trace=False)[0]


def estimate_time_ns():
    from concourse.timeline_sim import TimelineSim
    ts = TimelineSim(_get_nc(), trace=False, no_exec=True)
    return ts.simulate()


def kernel_timed(**inputs):
    out, _ = _run(inputs, trace=False)
    return out, estimate_time_ns()



# revision 3
# speedup vs baseline: 1.0868x; 1.0868x over previous
import sys

sys.path.insert(0, "/opt/trn_rl_repo")

import ml_dtypes
import numpy as np

import concourse.bass as bass
import concourse.tile as tile
from concourse import bacc, mybir
from concourse.bass_utils import run_bass_kernel_spmd

F32 = mybir.dt.float32
F32R = mybir.dt.float32r
BF16 = mybir.dt.bfloat16
AF = mybir.ActivationFunctionType

BATCH = 2
SEQ = 2048
D = 1024
NHEADS = 16
DK = 64
HPC = 4          # heads per core
NCORES = 8
THETA = 10000.0
CHUNK = 512
NCH = SEQ // CHUNK   # 4 chunks of queries
NBLK = SEQ // 128    # 16 key blocks


def _build_nc():
    nc = bacc.Bacc("TRN2", target_bir_lowering=False)
    XT = nc.declare_dram_parameter("XT", [128, 8, SEQ], BF16, isOutput=False)
    WT = nc.declare_dram_parameter("WT", [128, 8, 768], BF16, isOutput=False)
    COS = nc.declare_dram_parameter("COS", [128, SEQ], BF16, isOutput=False)
    SIN = nc.declare_dram_parameter("SIN", [128, SEQ], BF16, isOutput=False)
    WOT = nc.declare_dram_parameter("WOT", [128, 2, D], BF16, isOutput=False)
    INDT = nc.declare_dram_parameter("INDT", [128, 4], BF16, isOutput=False)
    I2 = nc.declare_dram_parameter("I2", [4, 128], BF16, isOutput=False)
    LNG = nc.declare_dram_parameter("LNG", [4, 1], F32, isOutput=False)
    TRI = nc.declare_dram_parameter("TRI", [128, 128], BF16, isOutput=False)
    OUT = nc.declare_dram_parameter("OUT", [SEQ, D], F32, isOutput=True)

    with tile.TileContext(nc) as tc:
        with (
            nc.allow_low_precision(reason="bf16 matmuls validated at 1e-2 rel err"),
            tc.tile_pool(name="cst", bufs=1) as cst,
            tc.tile_pool(name="xtp", bufs=2) as xtp,
            tc.tile_pool(name="tmp", bufs=10) as tmp,
            tc.tile_pool(name="expp", bufs=3) as expp,
            tc.tile_pool(name="bcp", bufs=2) as bcp,
            tc.tile_pool(name="ocp", bufs=2) as ocp,
            tc.tile_pool(name="ps", bufs=2, space="PSUM") as ps,
        ):
            wt_sb = cst.tile([128, 8, 768], BF16, tag="wt")
            cos_sb = cst.tile([128, SEQ], BF16, tag="cos")
            sin_sb = cst.tile([128, SEQ], BF16, tag="sin")
            wot_sb = cst.tile([128, 2, D], BF16, tag="wot")
            indt_sb = cst.tile([128, 4], BF16, tag="indt")
            i2_sb = cst.tile([4, 128], BF16, tag="i2")
            lng_sb = cst.tile([4, 1], F32, tag="lng")
            tri_sb = cst.tile([128, 128], BF16, tag="tri")
            q_sb = cst.tile([128, 2, SEQ], BF16, tag="q")
            k_sb = cst.tile([128, 2, SEQ], BF16, tag="k")
            v_sb = cst.tile([128, NBLK, HPC, 65], BF16, tag="v")
            ot_sb = cst.tile([128, 2, SEQ], BF16, tag="ot")
            kre_sb = cst.tile([128, NBLK * HPC], F32, tag="kre")
            dum_sb = cst.tile([1, 64], F32, tag="dum")

            def emit_xt(c):
                c0 = c * CHUNK
                xt_t = xtp.tile([128, 8, CHUNK], BF16, tag="xt", name=f"xt_{c}")
                nc.sync.dma_start(out=xt_t[:], in_=XT[:, :, c0:c0 + CHUNK])
                return xt_t

            # startup: first proj needs xt(0) + Q weight columns; issue those
            # DMAs ahead of everything else
            xt0 = emit_xt(0)
            nc.sync.dma_start(out=wt_sb[:, :, 0:256], in_=WT[:, :, 0:256])
            nc.sync.dma_start(out=indt_sb[:], in_=INDT[:])
            nc.sync.dma_start(out=i2_sb[:], in_=I2[:])
            nc.sync.dma_start(out=lng_sb[:].bitcast(F32R), in_=LNG[:].bitcast(F32R))
            nc.sync.dma_start(out=cos_sb[:], in_=COS[:])
            nc.sync.dma_start(out=sin_sb[:], in_=SIN[:])

            # ones column 64 of each v block for the denominator trick (data
            # cols are overwritten by the V projection); gpsimd keeps it off
            # the DVE queue
            nc.gpsimd.memset(dum_sb[:], 1.0)
            nc.gpsimd.memset(v_sb[:], 1.0)
            # warm the ln/exp act table before any real activation needs it
            dln = tmp.tile([1, 64], F32, tag="t", name="dln")
            nc.scalar.activation(dln[:], dum_sb[:], AF.Ln)

            def emit_proj_qk(c, qk, xt_t):
                c0 = c * CHUNK
                qoff = 256 * qk
                dst = q_sb if qk == 0 else k_sb

                pA = ps.tile([128, CHUNK], F32, tag="pp", name=f"pA_{qk}_{c}")
                for di in range(8):
                    nc.tensor.matmul(
                        pA,
                        lhsT=wt_sb[:, di, qoff:qoff + 128],
                        rhs=xt_t[:, di, :],
                        start=(di == 0), stop=(di == 7),
                    )
                pB = ps.tile([128, CHUNK], F32, tag="pp", name=f"pB_{qk}_{c}")
                for di in range(8):
                    nc.tensor.matmul(
                        pB,
                        lhsT=wt_sb[:, di, qoff + 128:qoff + 256],
                        rhs=xt_t[:, di, :],
                        start=(di == 0), stop=(di == 7),
                    )

                pAc = tmp.tile([128, CHUNK], BF16, tag="t", name=f"pAc_{qk}_{c}")
                nc.vector.tensor_copy(pAc[:], pA[:])
                pBc = tmp.tile([128, CHUNK], BF16, tag="t", name=f"pBc_{qk}_{c}")
                nc.vector.tensor_copy(pBc[:], pB[:])
                sqA = tmp.tile([128, CHUNK], BF16, tag="t", name=f"sqA_{qk}_{c}")
                nc.vector.tensor_mul(sqA[:], pAc[:], pAc[:])
                sqB = tmp.tile([128, CHUNK], BF16, tag="t", name=f"sqB_{qk}_{c}")
                nc.vector.tensor_mul(sqB[:], pBc[:], pBc[:])
                ssum = tmp.tile([128, CHUNK], BF16, tag="t", name=f"ssum_{qk}_{c}")
                nc.vector.tensor_add(ssum[:], sqA[:], sqB[:])

                cs = cos_sb[:, c0:c0 + CHUNK]
                sn = sin_sb[:, c0:c0 + CHUNK]

                if qk == 0:
                    # per-(head, position) 1/||q||: n2 -> exp(-.5 ln + ln g)
                    n2 = ps.tile([4, CHUNK], F32, tag="mm", name=f"n2_{c}")
                    nc.tensor.matmul(n2, lhsT=indt_sb[:], rhs=ssum[:],
                                     start=True, stop=True)
                    lnq = tmp.tile([4, CHUNK], F32, tag="t", name=f"lnq_{c}")
                    nc.scalar.activation(lnq[:], n2[:], AF.Ln)
                    rbq = tmp.tile([4, CHUNK], BF16, tag="t", name=f"rbq_{c}")
                    nc.scalar.activation(rbq[:], lnq[:], AF.Exp,
                                         bias=lng_sb[:], scale=-0.5)
                    rbp = ps.tile([128, CHUNK], F32, tag="mm", name=f"rbp_{c}")
                    nc.tensor.matmul(
                        rbp, lhsT=i2_sb[:], rhs=rbq[:], start=True, stop=True,
                    )
                    rb = tmp.tile([128, CHUNK], BF16, tag="t", name=f"rb_{c}")
                    nc.vector.tensor_copy(rb[:], rbp[:])
                    rbc = tmp.tile([128, CHUNK], BF16, tag="t", name=f"rbc_{c}")
                    nc.vector.tensor_mul(rbc[:], rb[:], cs)
                    rbs = tmp.tile([128, CHUNK], BF16, tag="t", name=f"rbs_{c}")
                    nc.vector.tensor_mul(rbs[:], rb[:], sn)
                else:
                    # per-key 1/||k||, transposed to [key, head] for use as
                    # the exp() scale operand
                    kn = ps.tile([128, 16], F32, tag="mm", name=f"kn_{c}")
                    for bb in range(4):
                        nc.tensor.matmul(
                            kn[:, 4 * bb:4 * bb + 4],
                            lhsT=ssum[:, bb * 128:bb * 128 + 128],
                            rhs=indt_sb[:], start=True, stop=True,
                        )
                    lnk = tmp.tile([128, 16], F32, tag="kt", name=f"lnk_{c}")
                    nc.scalar.activation(lnk[:], kn[:], AF.Ln)
                    nc.scalar.activation(kre_sb[:, 16 * c:16 * c + 16],
                                         lnk[:], AF.Exp, scale=-0.5)
                    rbc, rbs = cs, sn

                tac = tmp.tile([128, CHUNK], BF16, tag="t", name=f"tac_{qk}_{c}")
                nc.vector.tensor_mul(tac[:], pAc[:], rbc[:] if qk == 0 else cs)
                tas = tmp.tile([128, CHUNK], BF16, tag="t", name=f"tas_{qk}_{c}")
                nc.vector.tensor_mul(tas[:], pAc[:], rbs[:] if qk == 0 else sn)
                tbc = tmp.tile([128, CHUNK], BF16, tag="t", name=f"tbc_{qk}_{c}")
                nc.vector.tensor_mul(tbc[:], pBc[:], rbc[:] if qk == 0 else cs)
                tbs = tmp.tile([128, CHUNK], BF16, tag="t", name=f"tbs_{qk}_{c}")
                nc.vector.tensor_mul(tbs[:], pBc[:], rbs[:] if qk == 0 else sn)

                for h in range(HPC):
                    po = (h % 2) * 64
                    ti = h // 2
                    hs = 32 * h
                    nc.vector.tensor_sub(
                        dst[po:po + 32, ti, c0:c0 + CHUNK],
                        tac[hs:hs + 32, :], tbs[hs:hs + 32, :])
                    nc.vector.tensor_add(
                        dst[po + 32:po + 64, ti, c0:c0 + CHUNK],
                        tas[hs:hs + 32, :], tbc[hs:hs + 32, :])

            def emit_proj_v(c, half, xt_t):
                for bb in (2 * half, 2 * half + 1):
                    nb = 4 * c + bb
                    vps = ps.tile([128, HPC, 64], F32, tag="mm", name=f"vps_{nb}")
                    for di in range(8):
                        nc.tensor.matmul(
                            vps,
                            lhsT=xt_t[:, di, bb * 128:bb * 128 + 128],
                            rhs=wt_sb[:, di, 512:768],
                            start=(di == 0), stop=(di == 7),
                        )
                    nc.vector.tensor_copy(v_sb[:, nb, :, 0:64], vps[:])

            def emit_attn_head(c, h):
                c0 = c * CHUNK
                njb = 4 * (c + 1)
                po = (h % 2) * 64
                ti = h // 2
                av = ps.tile([65, CHUNK], F32, tag="av", name=f"av_{h}_{c}")

                def issue_sc(jb):
                    bb = jb - 4 * c
                    lo = 128 * bb if bb > 0 else 0
                    sc = ps.tile([128, CHUNK], F32, tag="sc", name=f"sc_{h}_{c}_{jb}")
                    nc.tensor.matmul(
                        sc[:, lo:],
                        lhsT=k_sb[po:po + 64, ti, jb * 128:jb * 128 + 128],
                        rhs=q_sb[po:po + 64, ti, c0 + lo:c0 + CHUNK],
                        start=True, stop=True,
                    )
                    return sc, lo

                # stagger: issue sc for jb+1 before draining jb so PE keeps
                # ahead of ACT's exp stream
                cur = issue_sc(0)
                for jb in range(njb):
                    nxt = issue_sc(jb + 1) if jb + 1 < njb else None
                    sc, lo = cur
                    diag = jb >= 4 * c
                    ex = expp.tile([128, CHUNK], BF16, tag="ex",
                                   name=f"ex_{h}_{c}_{jb}")
                    nc.scalar.activation(
                        ex[:, lo:], sc[:, lo:], AF.Exp,
                        scale=kre_sb[:, 4 * jb + h:4 * jb + h + 1])
                    if diag:
                        # zero the strictly-upper triangle of the 128-col
                        # window at the causal boundary
                        nc.vector.tensor_mul(
                            ex[:, lo:lo + 128], ex[:, lo:lo + 128], tri_sb[:])
                    nc.tensor.matmul(
                        av[:, lo:], lhsT=v_sb[:, jb, h, :], rhs=ex[:, lo:],
                        start=(jb == 0), stop=(jb == njb - 1),
                        skip_group_check=True,
                    )
                    cur = nxt

                srec = bcp.tile([1, CHUNK], F32, tag="srec", name=f"srec_{h}_{c}")
                nc.vector.reciprocal(srec[:].bitcast(F32R), av[64:65, :])
                rb2 = bcp.tile([64, CHUNK], F32, tag="rb2", name=f"rb2_{h}_{c}")
                nc.gpsimd.partition_broadcast(rb2[:], srec[0:1, :])
                nc.vector.tensor_mul(
                    ot_sb[po:po + 64, ti, c0:c0 + CHUNK],
                    av[0:64, :], rb2[:])

            def emit_outproj(c):
                for bb in range(4):
                    nb = 4 * c + bb
                    for oc in range(2):
                        wo = ps.tile([128, CHUNK], F32, tag="mm", name=f"wo_{nb}_{oc}")
                        for ti in range(2):
                            nc.tensor.matmul(
                                wo,
                                lhsT=ot_sb[:, ti, nb * 128:nb * 128 + 128],
                                rhs=wot_sb[:, ti, oc * CHUNK:oc * CHUNK + CHUNK],
                                start=(ti == 0), stop=(ti == 1),
                            )
                        ob = ocp.tile([128, CHUNK], F32, tag="ob", name=f"ob_{nb}_{oc}")
                        nc.vector.tensor_copy(ob[:], wo[:])
                        nc.sync.dma_start(
                            out=OUT[nb * 128:nb * 128 + 128, oc * CHUNK:oc * CHUNK + CHUNK],
                            in_=ob[:])

            # warm-up row: chunk 0 projections, with remaining const DMAs
            # slotted where they are first needed
            emit_proj_qk(0, 0, xt0)
            nc.sync.dma_start(out=wt_sb[:, :, 256:512], in_=WT[:, :, 256:512])
            emit_proj_qk(0, 1, xt0)
            nc.sync.dma_start(out=wt_sb[:, :, 512:768], in_=WT[:, :, 512:768])
            emit_proj_v(0, 0, xt0)
            nc.sync.dma_start(out=tri_sb[:], in_=TRI[:])
            emit_proj_v(0, 1, xt0)
            nc.sync.dma_start(out=wot_sb[:], in_=WOT[:])

            # steady rows: interleave chunk c's projections (PE/DVE-heavy)
            # with chunk c-1's attention heads (ACT-heavy) so no engine
            # queue starves and the PE p-state stays ramped
            for c in range(1, NCH):
                xtc = emit_xt(c)
                emit_proj_qk(c, 0, xtc)
                emit_attn_head(c - 1, 0)
                emit_proj_qk(c, 1, xtc)
                emit_attn_head(c - 1, 1)
                emit_proj_v(c, 0, xtc)
                emit_attn_head(c - 1, 2)
                emit_proj_v(c, 1, xtc)
                emit_attn_head(c - 1, 3)
                emit_outproj(c - 1)

            for h in range(HPC):
                emit_attn_head(NCH - 1, h)
            emit_outproj(NCH - 1)
    return nc


_NC = None


def _get_nc():
    global _NC
    if _NC is None:
        _NC = _build_nc()
        _NC.finalize()
    return _NC


def _shared_tables(token_positions):
    freqs = np.arange(0, DK, 2, dtype=np.float64)
    inv_theta = THETA ** (-freqs / DK)                      # [32]
    pos = token_positions.astype(np.float64)
    ang = inv_theta[:, None] * pos[None, :]                 # [32, SEQ]
    cos_t = np.ascontiguousarray(
        np.tile(np.cos(ang), (4, 1))).astype(ml_dtypes.bfloat16)
    sin_t = np.ascontiguousarray(
        np.tile(np.sin(ang), (4, 1))).astype(ml_dtypes.bfloat16)

    indt = np.zeros((128, 4), dtype=np.float32)
    for j in range(4):
        indt[32 * j:32 * j + 32, j] = 1.0
    i2 = np.ascontiguousarray(indt.T).astype(ml_dtypes.bfloat16)

    p_i = np.arange(128)[:, None]
    t_i = np.arange(128)[None, :]
    tri = (p_i <= t_i).astype(ml_dtypes.bfloat16)
    return cos_t, sin_t, indt.astype(ml_dtypes.bfloat16), i2, tri


def _core_inputs(c, x, W_QKV, W_O, qk_scale, shared):
    cos_t, sin_t, indt, i2, tri = shared
    b = c // 4
    a = c % 4
    heads = [4 * a + i for i in range(HPC)]

    qA = [64 * h + 2 * t for h in heads for t in range(32)]
    qB = [64 * h + 2 * t + 1 for h in heads for t in range(32)]
    kA = [1024 + r for r in qA]
    kB = [1024 + r for r in qB]
    vr = [2048 + 64 * h + j for h in heads for j in range(DK)]
    rows = qA + qB + kA + kB + vr
    wt = np.ascontiguousarray(
        W_QKV[rows, :].T.reshape(8, 128, 768).transpose(1, 0, 2)
    ).astype(ml_dtypes.bfloat16)

    vcols = [64 * h + j for h in heads for j in range(DK)]
    wot = np.ascontiguousarray(
        W_O[:, vcols].T.reshape(2, 128, D).transpose(1, 0, 2)
    ).astype(ml_dtypes.bfloat16)

    xt = np.ascontiguousarray(
        x[b].T.reshape(8, 128, SEQ).transpose(1, 0, 2)
    ).astype(ml_dtypes.bfloat16)

    lng = np.log(qk_scale[heads].astype(np.float64)).astype(
        np.float32).reshape(4, 1)

    return {
        "XT": xt, "WT": wt, "COS": cos_t, "SIN": sin_t, "WOT": wot,
        "INDT": indt, "I2": i2, "LNG": lng, "TRI": tri,
    }


def _run(inputs, trace=False):
    x = np.asarray(inputs["x"], dtype=np.float32)
    token_positions = np.asarray(inputs["token_positions"])
    W_QKV = np.asarray(inputs["W_QKV"], dtype=np.float32)
    W_O = np.asarray(inputs["W_O"], dtype=np.float32)
    qk_scale = np.asarray(inputs["qk_scale"], dtype=np.float32)

    shared = _shared_tables(token_positions)
    nc = _get_nc()
    in_maps = [_core_inputs(c, x, W_QKV, W_O, qk_scale, shared)
               for c in range(NCORES)]
    core_ids = list(range(NCORES))
    kw = {}
    if trace:
        kw = dict(trace=True, trace_cores=core_ids)
    res = run_bass_kernel_spmd(nc, in_maps, core_ids, **kw)
    parts = [np.asarray(r["OUT"], dtype=np.float32) for r in res.results]
    out = np.stack([
        parts[0] + parts[1] + parts[2] + parts[3],
        parts[4] + parts[5] + parts[6] + parts[7],
    ]).astype(np.float32)
    return out, getattr(res, "exec_time_ns", None)


def kernel(**inputs):
    return _run(inputs, trace=False)[0]


def estimate_time_ns():
    from concourse.timeline_sim import TimelineSim
    ts = TimelineSim(_get_nc(), trace=False, no_exec=True)
    return ts.simulate()


def kernel_timed(**inputs):
    out, _ = _run(inputs, trace=False)
    return out, estimate_time_ns()


# revision 5
# speedup vs baseline: 1.2793x; 1.1770x over previous
import sys

sys.path.insert(0, "/opt/trn_rl_repo")

import ml_dtypes
import numpy as np

import concourse.bass as bass
import concourse.tile as tile
from concourse import bacc, mybir
from concourse.bass_utils import run_bass_kernel_spmd

F32 = mybir.dt.float32
F32R = mybir.dt.float32r
BF16 = mybir.dt.bfloat16
AF = mybir.ActivationFunctionType

BATCH = 2
SEQ = 2048
D = 1024
NHEADS = 16
DK = 64
HPC = 4          # heads per core
NCORES = 8
THETA = 10000.0
CHUNK = 512
NCH = SEQ // CHUNK   # 4 chunks of queries
NBLK = SEQ // 128    # 16 key blocks


def _build_nc():
    nc = bacc.Bacc("TRN2", target_bir_lowering=False)
    XT = nc.declare_dram_parameter("XT", [128, 8, SEQ], BF16, isOutput=False)
    WT = nc.declare_dram_parameter("WT", [128, 8, 768], BF16, isOutput=False)
    COS = nc.declare_dram_parameter("COS", [128, SEQ], BF16, isOutput=False)
    SIN = nc.declare_dram_parameter("SIN", [128, SEQ], BF16, isOutput=False)
    WOT = nc.declare_dram_parameter("WOT", [128, 2, D], BF16, isOutput=False)
    INDT = nc.declare_dram_parameter("INDT", [128, 4], BF16, isOutput=False)
    I2 = nc.declare_dram_parameter("I2", [4, 128], BF16, isOutput=False)
    LNG = nc.declare_dram_parameter("LNG", [4, 1], F32, isOutput=False)
    TRI = nc.declare_dram_parameter("TRI", [128, 128], BF16, isOutput=False)
    OUT = nc.declare_dram_parameter("OUT", [SEQ, D], F32, isOutput=True)

    with tile.TileContext(nc) as tc:
        with (
            nc.allow_low_precision(reason="bf16 matmuls validated at 1e-2 rel err"),
            tc.tile_pool(name="cst", bufs=1) as cst,
            tc.tile_pool(name="xtp", bufs=2) as xtp,
            tc.tile_pool(name="tmp", bufs=10) as tmp,
            tc.tile_pool(name="expp", bufs=3) as expp,
            tc.tile_pool(name="bcp", bufs=2) as bcp,
            tc.tile_pool(name="ocp", bufs=2) as ocp,
            tc.tile_pool(name="ps", bufs=2, space="PSUM") as ps,
        ):
            wt_sb = cst.tile([128, 8, 768], BF16, tag="wt")
            cos_sb = cst.tile([128, SEQ], BF16, tag="cos")
            sin_sb = cst.tile([128, SEQ], BF16, tag="sin")
            wot_sb = cst.tile([128, 2, D], BF16, tag="wot")
            indt_sb = cst.tile([128, 4], BF16, tag="indt")
            i2_sb = cst.tile([4, 128], BF16, tag="i2")
            lng_sb = cst.tile([4, 1], F32, tag="lng")
            tri_sb = cst.tile([128, 128], BF16, tag="tri")
            q_sb = cst.tile([128, 2, SEQ], BF16, tag="q")
            k_sb = cst.tile([128, 2, SEQ], BF16, tag="k")
            v_sb = cst.tile([128, NBLK, HPC, 65], BF16, tag="v")
            ot_sb = cst.tile([128, 2, SEQ], BF16, tag="ot")
            kre_sb = cst.tile([128, NBLK * HPC], F32, tag="kre")
            dum_sb = cst.tile([1, 64], F32, tag="dum")

            xts = {}

            def emit_xt(c):
                c0 = c * CHUNK
                xt_t = xtp.tile([128, 8, CHUNK], BF16, tag="xt", name=f"xt_{c}")
                nc.sync.dma_start(out=xt_t[:, 0:4, :], in_=XT[:, 0:4, c0:c0 + CHUNK])
                nc.sync.dma_start(out=xt_t[:, 4:8, :], in_=XT[:, 4:8, c0:c0 + CHUNK])
                xts[c] = xt_t

            # startup: first proj needs xt(0) + Q weight columns; issue those
            # DMAs ahead of everything else, smallest-first so the first
            # matmuls can start early
            emit_xt(0)
            nc.sync.dma_start(out=wt_sb[:, :, 0:128], in_=WT[:, :, 0:128])
            nc.sync.dma_start(out=wt_sb[:, :, 128:256], in_=WT[:, :, 128:256])
            nc.sync.dma_start(out=cos_sb[:], in_=COS[:])
            nc.sync.dma_start(out=sin_sb[:], in_=SIN[:])
            nc.sync.dma_start(out=indt_sb[:], in_=INDT[:])
            nc.sync.dma_start(out=i2_sb[:], in_=I2[:])
            nc.sync.dma_start(out=lng_sb[:].bitcast(F32R), in_=LNG[:].bitcast(F32R))
            nc.sync.dma_start(out=wt_sb[:, :, 256:512], in_=WT[:, :, 256:512])
            nc.sync.dma_start(out=wt_sb[:, :, 512:768], in_=WT[:, :, 512:768])
            nc.sync.dma_start(out=tri_sb[:], in_=TRI[:])
            nc.sync.dma_start(out=wot_sb[:], in_=WOT[:])
            emit_xt(1)

            # ones column 64 of each v block for the denominator trick (data
            # cols are overwritten by the V projection); gpsimd keeps it off
            # the DVE queue
            nc.gpsimd.memset(dum_sb[:], 1.0)
            nc.gpsimd.memset(v_sb[:], 1.0)
            # pre-load the combined ln+exp act table so the table-load pass
            # (greedy first-fit per function) never has to swap tables
            nc.scalar.add_instruction(mybir.InstLoadActFuncSet(
                name=nc.get_next_instruction_name(),
                act_func_set_id=6, engine=mybir.EngineType.Activation))
            dln = tmp.tile([1, 64], F32, tag="t", name="dln")
            nc.scalar.activation(dln[:], dum_sb[:], AF.Ln)

            def projqk_units(c, qk, units):
                c0 = c * CHUNK
                qoff = 256 * qk
                dst = q_sb if qk == 0 else k_sb
                st = {}

                def mk_mm(which, di, qo):
                    def u(which=which, di=di, qo=qo):
                        if di == 0:
                            st[which] = ps.tile(
                                [128, CHUNK], F32, tag="pp",
                                name=f"p{which}_{qk}_{c}")
                        nc.tensor.matmul(
                            st[which],
                            lhsT=wt_sb[:, di, qo:qo + 128],
                            rhs=xts[c][:, di, :],
                            start=(di == 0), stop=(di == 7),
                        )
                    return u

                for di in range(8):
                    units.append(mk_mm("A", di, qoff))

                def uA():
                    pAc = tmp.tile([128, CHUNK], BF16, tag="t",
                                   name=f"pAc_{qk}_{c}")
                    nc.scalar.copy(pAc[:], st["A"][:])
                    sqA = tmp.tile([128, CHUNK], BF16, tag="t",
                                   name=f"sqA_{qk}_{c}")
                    nc.vector.tensor_mul(sqA[:], pAc[:], pAc[:])
                    st["Ac"], st["sqA"] = pAc, sqA
                units.append(uA)

                for di in range(8):
                    units.append(mk_mm("B", di, qoff + 128))

                def uB():
                    pBc = tmp.tile([128, CHUNK], BF16, tag="t",
                                   name=f"pBc_{qk}_{c}")
                    nc.scalar.copy(pBc[:], st["B"][:])
                    sqB = tmp.tile([128, CHUNK], BF16, tag="t",
                                   name=f"sqB_{qk}_{c}")
                    nc.vector.tensor_mul(sqB[:], pBc[:], pBc[:])
                    ssum = tmp.tile([128, CHUNK], BF16, tag="t",
                                    name=f"ssum_{qk}_{c}")
                    nc.vector.tensor_add(ssum[:], st["sqA"][:], sqB[:])
                    st["Bc"], st["ssum"] = pBc, ssum
                units.append(uB)

                cs = cos_sb[:, c0:c0 + CHUNK]
                sn = sin_sb[:, c0:c0 + CHUNK]

                if qk == 0:
                    # per-(head, position) g/||q||: n2 -> exp(-.5 ln + ln g)
                    def uN():
                        n2 = ps.tile([4, CHUNK], F32, tag="mm", name=f"n2_{c}")
                        nc.tensor.matmul(n2, lhsT=indt_sb[:], rhs=st["ssum"][:],
                                         start=True, stop=True)
                        lnq = tmp.tile([4, CHUNK], F32, tag="t", name=f"lnq_{c}")
                        nc.scalar.activation(lnq[:], n2[:], AF.Ln)
                        rbq = tmp.tile([4, CHUNK], BF16, tag="t", name=f"rbq_{c}")
                        nc.scalar.activation(rbq[:], lnq[:], AF.Exp,
                                             bias=lng_sb[:], scale=-0.5)
                        st["rbq"] = rbq
                    units.append(uN)

                    def uBC():
                        rbp = ps.tile([128, CHUNK], F32, tag="mm", name=f"rbp_{c}")
                        nc.tensor.matmul(rbp, lhsT=i2_sb[:], rhs=st["rbq"][:],
                                         start=True, stop=True)
                        rb = tmp.tile([128, CHUNK], BF16, tag="t", name=f"rb_{c}")
                        nc.scalar.copy(rb[:], rbp[:])
                        rbc = tmp.tile([128, CHUNK], BF16, tag="t", name=f"rbc_{c}")
                        nc.vector.tensor_mul(rbc[:], rb[:], cs)
                        rbs = tmp.tile([128, CHUNK], BF16, tag="t", name=f"rbs_{c}")
                        nc.vector.tensor_mul(rbs[:], rb[:], sn)
                        st["rbc"], st["rbs"] = rbc, rbs
                    units.append(uBC)
                else:
                    # per-key 1/||k||, transposed to [key, head] for use as
                    # the exp() scale operand
                    def uKN():
                        kn = ps.tile([128, 16], F32, tag="mm", name=f"kn_{c}")
                        for bb in range(4):
                            nc.tensor.matmul(
                                kn[:, 4 * bb:4 * bb + 4],
                                lhsT=st["ssum"][:, bb * 128:bb * 128 + 128],
                                rhs=indt_sb[:], start=True, stop=True,
                            )
                        lnk = tmp.tile([128, 16], F32, tag="kt", name=f"lnk_{c}")
                        nc.scalar.activation(lnk[:], kn[:], AF.Ln)
                        nc.scalar.activation(kre_sb[:, 16 * c:16 * c + 16],
                                             lnk[:], AF.Exp, scale=-0.5)
                    units.append(uKN)

                def uProd():
                    pc = st["rbc"][:] if qk == 0 else cs
                    pss = st["rbs"][:] if qk == 0 else sn
                    for nm, src, mulby in (("tac", "Ac", pc), ("tas", "Ac", pss),
                                           ("tbc", "Bc", pc), ("tbs", "Bc", pss)):
                        t = tmp.tile([128, CHUNK], BF16, tag="t",
                                     name=f"{nm}_{qk}_{c}")
                        nc.vector.tensor_mul(t[:], st[src][:], mulby)
                        st[nm] = t
                units.append(uProd)

                def uComb():
                    for h in range(HPC):
                        po = (h % 2) * 64
                        ti = h // 2
                        hs = 32 * h
                        nc.vector.tensor_sub(
                            dst[po:po + 32, ti, c0:c0 + CHUNK],
                            st["tac"][hs:hs + 32, :], st["tbs"][hs:hs + 32, :])
                        nc.vector.tensor_add(
                            dst[po + 32:po + 64, ti, c0:c0 + CHUNK],
                            st["tas"][hs:hs + 32, :], st["tbc"][hs:hs + 32, :])
                units.append(uComb)

            def projv_units(c, units):
                for bb in range(4):
                    nb = 4 * c + bb
                    st = {}

                    def u1(bb=bb, nb=nb, st=st):
                        st["v"] = ps.tile([128, HPC, 64], F32, tag="mm",
                                          name=f"vps_{nb}")
                        for di in range(4):
                            nc.tensor.matmul(
                                st["v"],
                                lhsT=xts[c][:, di, bb * 128:bb * 128 + 128],
                                rhs=wt_sb[:, di, 512:768],
                                start=(di == 0), stop=False,
                            )

                    def u2(bb=bb, nb=nb, st=st):
                        for di in range(4, 8):
                            nc.tensor.matmul(
                                st["v"],
                                lhsT=xts[c][:, di, bb * 128:bb * 128 + 128],
                                rhs=wt_sb[:, di, 512:768],
                                start=False, stop=(di == 7),
                            )
                        nc.vector.tensor_copy(v_sb[:, nb, :, 0:64], st["v"][:])
                    units.append(u1)
                    units.append(u2)

            def outproj_units(c, units):
                for bb in range(4):
                    nb = 4 * c + bb
                    for oc in range(2):
                        def u(nb=nb, oc=oc):
                            wo = ps.tile([128, CHUNK], F32, tag="mm",
                                         name=f"wo_{nb}_{oc}")
                            for ti in range(2):
                                nc.tensor.matmul(
                                    wo,
                                    lhsT=ot_sb[:, ti, nb * 128:nb * 128 + 128],
                                    rhs=wot_sb[:, ti, oc * CHUNK:oc * CHUNK + CHUNK],
                                    start=(ti == 0), stop=(ti == 1),
                                )
                            ob = ocp.tile([128, CHUNK], F32, tag="ob",
                                          name=f"ob_{nb}_{oc}")
                            nc.vector.tensor_copy(ob[:], wo[:])
                            nc.sync.dma_start(
                                out=OUT[nb * 128:nb * 128 + 128,
                                        oc * CHUNK:oc * CHUNK + CHUNK],
                                in_=ob[:])
                        units.append(u)

            def mk_fill(units):
                state = {"i": 0}

                def fill(n):
                    while n > 0 and state["i"] < len(units):
                        units[state["i"]]()
                        state["i"] += 1
                        n -= 1
                return fill

            def emit_attn_head(c, h, fill):
                c0 = c * CHUNK
                njb = 4 * (c + 1)
                po = (h % 2) * 64
                ti = h // 2
                av = ps.tile([65, CHUNK], F32, tag="av", name=f"av_{h}_{c}")

                def issue_sc(jb):
                    bb = jb - 4 * c
                    lo = 128 * bb if bb > 0 else 0
                    sc = ps.tile([128, CHUNK], F32, tag="sc",
                                 name=f"sc_{h}_{c}_{jb}")
                    nc.tensor.matmul(
                        sc[:, lo:],
                        lhsT=k_sb[po:po + 64, ti, jb * 128:jb * 128 + 128],
                        rhs=q_sb[po:po + 64, ti, c0 + lo:c0 + CHUNK],
                        start=True, stop=True,
                    )
                    return sc, lo

                # stagger: issue sc for jb+1 before draining jb so PE keeps
                # ahead of ACT's exp stream; fill PE bubbles with proj work
                cur = issue_sc(0)
                for jb in range(njb):
                    nxt = issue_sc(jb + 1) if jb + 1 < njb else None
                    sc, lo = cur
                    diag = jb >= 4 * c
                    ex = expp.tile([128, CHUNK], BF16, tag="ex",
                                   name=f"ex_{h}_{c}_{jb}")
                    nc.scalar.activation(
                        ex[:, lo:], sc[:, lo:], AF.Exp,
                        scale=kre_sb[:, 4 * jb + h:4 * jb + h + 1])
                    if diag:
                        # zero the strictly-upper triangle of the 128-col
                        # window at the causal boundary
                        nc.vector.tensor_mul(
                            ex[:, lo:lo + 128], ex[:, lo:lo + 128], tri_sb[:])
                    nc.tensor.matmul(
                        av[:, lo:], lhsT=v_sb[:, jb, h, :], rhs=ex[:, lo:],
                        start=(jb == 0), stop=(jb == njb - 1),
                        skip_group_check=True,
                    )
                    fill(1)
                    cur = nxt

                srec = bcp.tile([1, CHUNK], F32, tag="srec", name=f"srec_{h}_{c}")
                nc.vector.reciprocal(srec[:].bitcast(F32R), av[64:65, :])
                rb2 = bcp.tile([64, CHUNK], F32, tag="rb2", name=f"rb2_{h}_{c}")
                nc.gpsimd.partition_broadcast(rb2[:], srec[0:1, :])
                nc.vector.tensor_mul(
                    ot_sb[po:po + 64, ti, c0:c0 + CHUNK],
                    av[0:64, :], rb2[:])

            # warm-up: chunk 0 projections run back-to-back (no attention
            # yet to interleave with)
            units0 = []
            projqk_units(0, 0, units0)
            projqk_units(0, 1, units0)
            projv_units(0, units0)
            for u in units0:
                u()

            # steady phases: attention for chunk c-1 interleaved, at matmul
            # granularity, with chunk c's projections and chunk c-2's
            # output projection so PE never starves while ACT streams exps
            for c in range(1, NCH):
                units = []
                if c + 1 < NCH:
                    units.append(lambda c=c: emit_xt(c + 1))
                if c >= 2:
                    outproj_units(c - 2, units)
                projqk_units(c, 0, units)
                projqk_units(c, 1, units)
                projv_units(c, units)
                fill = mk_fill(units)
                for h in range(HPC):
                    emit_attn_head(c - 1, h, fill)
                    fill(3)
                fill(len(units))

            units = []
            outproj_units(NCH - 2, units)
            fill = mk_fill(units)
            for h in range(HPC):
                emit_attn_head(NCH - 1, h, fill)
                fill(3)
            fill(len(units))
            unitsF = []
            outproj_units(NCH - 1, unitsF)
            for u in unitsF:
                u()
    return nc


_NC = None


def _get_nc():
    global _NC
    if _NC is None:
        _NC = _build_nc()
        _NC.finalize()
    return _NC


def _shared_tables(token_positions):
    freqs = np.arange(0, DK, 2, dtype=np.float64)
    inv_theta = THETA ** (-freqs / DK)                      # [32]
    pos = token_positions.astype(np.float64)
    ang = inv_theta[:, None] * pos[None, :]                 # [32, SEQ]
    cos_t = np.ascontiguousarray(
        np.tile(np.cos(ang), (4, 1))).astype(ml_dtypes.bfloat16)
    sin_t = np.ascontiguousarray(
        np.tile(np.sin(ang), (4, 1))).astype(ml_dtypes.bfloat16)

    indt = np.zeros((128, 4), dtype=np.float32)
    for j in range(4):
        indt[32 * j:32 * j + 32, j] = 1.0
    i2 = np.ascontiguousarray(indt.T).astype(ml_dtypes.bfloat16)

    p_i = np.arange(128)[:, None]
    t_i = np.arange(128)[None, :]
    tri = (p_i <= t_i).astype(ml_dtypes.bfloat16)
    return cos_t, sin_t, indt.astype(ml_dtypes.bfloat16), i2, tri


def _core_inputs(c, x, W_QKV, W_O, qk_scale, shared):
    cos_t, sin_t, indt, i2, tri = shared
    b = c // 4
    a = c % 4
    heads = [4 * a + i for i in range(HPC)]

    qA = [64 * h + 2 * t for h in heads for t in range(32)]
    qB = [64 * h + 2 * t + 1 for h in heads for t in range(32)]
    kA = [1024 + r for r in qA]
    kB = [1024 + r for r in qB]
    vr = [2048 + 64 * h + j for h in heads for j in range(DK)]
    rows = qA + qB + kA + kB + vr
    wt = np.ascontiguousarray(
        W_QKV[rows, :].T.reshape(8, 128, 768).transpose(1, 0, 2)
    ).astype(ml_dtypes.bfloat16)

    vcols = [64 * h + j for h in heads for j in range(DK)]
    wot = np.ascontiguousarray(
        W_O[:, vcols].T.reshape(2, 128, D).transpose(1, 0, 2)
    ).astype(ml_dtypes.bfloat16)

    xt = np.ascontiguousarray(
        x[b].T.reshape(8, 128, SEQ).transpose(1, 0, 2)
    ).astype(ml_dtypes.bfloat16)

    lng = np.log(qk_scale[heads].astype(np.float64)).astype(
        np.float32).reshape(4, 1)

    return {
        "XT": xt, "WT": wt, "COS": cos_t, "SIN": sin_t, "WOT": wot,
        "INDT": indt, "I2": i2, "LNG": lng, "TRI": tri,
    }


def _run(inputs, trace=False):
    x = np.asarray(inputs["x"], dtype=np.float32)
    token_positions = np.asarray(inputs["token_positions"])
    W_QKV = np.asarray(inputs["W_QKV"], dtype=np.float32)
    W_O = np.asarray(inputs["W_O"], dtype=np.float32)
    qk_scale = np.asarray(inputs["qk_scale"], dtype=np.float32)

    shared = _shared_tables(token_positions)
    nc = _get_nc()
    in_maps = [_core_inputs(c, x, W_QKV, W_O, qk_scale, shared)
               for c in range(NCORES)]
    core_ids = list(range(NCORES))
    kw = {}
    if trace:
        kw = dict(trace=True, trace_cores=core_ids)
    res = run_bass_kernel_spmd(nc, in_maps, core_ids, **kw)
    parts = [np.asarray(r["OUT"], dtype=np.float32) for r in res.results]
    out = np.stack([
        parts[0] + parts[1] + parts[2] + parts[3],
        parts[4] + parts[5] + parts[6] + parts[7],
    ]).astype(np.float32)
    return out, getattr(res, "exec_time_ns", None)


def kernel(**inputs):
    return _run(inputs, trace=False)[0]


def estimate_time_ns():
    from concourse.timeline_sim import TimelineSim
    ts = TimelineSim(_get_nc(), trace=False, no_exec=True)
    return ts.simulate()


def kernel_timed(**inputs):
    out, _ = _run(inputs, trace=False)
    return out, estimate_time_ns()


# revision 8
# speedup vs baseline: 1.3215x; 1.0331x over previous
import sys

sys.path.insert(0, "/opt/trn_rl_repo")

import ml_dtypes
import numpy as np

import concourse.bass as bass
import concourse.tile as tile
from concourse import bacc, mybir
from concourse.bass_utils import run_bass_kernel_spmd

F32 = mybir.dt.float32
F32R = mybir.dt.float32r
BF16 = mybir.dt.bfloat16
AF = mybir.ActivationFunctionType

BATCH = 2
SEQ = 2048
D = 1024
NHEADS = 16
DK = 64
HPC = 4          # heads per core
NCORES = 8
THETA = 10000.0
CHUNK = 512
NCH = SEQ // CHUNK   # 4 chunks of queries
NBLK = SEQ // 128    # 16 key blocks


def _build_nc():
    nc = bacc.Bacc("TRN2", target_bir_lowering=False)
    XT = nc.declare_dram_parameter("XT", [128, 8, SEQ], BF16, isOutput=False)
    WT = nc.declare_dram_parameter("WT", [128, 8, 768], BF16, isOutput=False)
    COS = nc.declare_dram_parameter("COS", [128, SEQ], BF16, isOutput=False)
    SIN = nc.declare_dram_parameter("SIN", [128, SEQ], BF16, isOutput=False)
    WOT = nc.declare_dram_parameter("WOT", [128, 2, D], BF16, isOutput=False)
    INDT = nc.declare_dram_parameter("INDT", [128, 4], BF16, isOutput=False)
    I2 = nc.declare_dram_parameter("I2", [4, 128], BF16, isOutput=False)
    LNG = nc.declare_dram_parameter("LNG", [4, 1], F32, isOutput=False)
    TRI = nc.declare_dram_parameter("TRI", [128, 128], BF16, isOutput=False)
    OUT = nc.declare_dram_parameter("OUT", [SEQ, D], F32, isOutput=True)

    with tile.TileContext(nc) as tc:
        with (
            nc.allow_low_precision(reason="bf16 matmuls validated at 1e-2 rel err"),
            tc.tile_pool(name="cst", bufs=1) as cst,
            tc.tile_pool(name="xtp", bufs=2) as xtp,
            tc.tile_pool(name="tmp", bufs=10) as tmp,
            tc.tile_pool(name="expp", bufs=3) as expp,
            tc.tile_pool(name="bcp", bufs=2) as bcp,
            tc.tile_pool(name="ocp", bufs=2) as ocp,
            tc.tile_pool(name="ps", bufs=2, space="PSUM") as ps,
        ):
            wt_sb = cst.tile([128, 8, 768], BF16, tag="wt")
            cos_sb = cst.tile([128, SEQ], BF16, tag="cos")
            sin_sb = cst.tile([128, SEQ], BF16, tag="sin")
            wot_sb = cst.tile([128, 2, D], BF16, tag="wot")
            indt_sb = cst.tile([128, 4], BF16, tag="indt")
            i2_sb = cst.tile([4, 128], BF16, tag="i2")
            lng_sb = cst.tile([4, 1], F32, tag="lng")
            tri_sb = cst.tile([128, 128], BF16, tag="tri")
            q_sb = cst.tile([128, 2, SEQ], BF16, tag="q")
            k_sb = cst.tile([128, 2, SEQ], BF16, tag="k")
            v_sb = cst.tile([128, NBLK, HPC, 65], BF16, tag="v")
            ot_sb = cst.tile([128, 2, SEQ], BF16, tag="ot")
            kre_sb = cst.tile([128, NBLK * HPC], F32, tag="kre")
            dum_sb = cst.tile([1, 64], F32, tag="dum")

            xts = {}

            def emit_xt(c):
                c0 = c * CHUNK
                xt_t = xtp.tile([128, 8, CHUNK], BF16, tag="xt", name=f"xt_{c}")
                nc.sync.dma_start(out=xt_t[:, 0:4, :], in_=XT[:, 0:4, c0:c0 + CHUNK])
                nc.sync.dma_start(out=xt_t[:, 4:8, :], in_=XT[:, 4:8, c0:c0 + CHUNK])
                xts[c] = xt_t

            # startup: first proj needs xt(0) di-slices + Q weight columns;
            # spread the first pieces across engine DGE queues (SP / Pool /
            # ACT / DVE run their DMAs concurrently) so the first matmul can
            # start as early as possible
            xt0 = xtp.tile([128, 8, CHUNK], BF16, tag="xt", name="xt_0")
            xts[0] = xt0
            nc.sync.dma_start(out=wt_sb[:, :, 0:128], in_=WT[:, :, 0:128])
            nc.gpsimd.dma_start(out=xt0[:, 0:2, :], in_=XT[:, 0:2, 0:CHUNK])
            nc.scalar.dma_start(out=xt0[:, 2:5, :], in_=XT[:, 2:5, 0:CHUNK])
            nc.gpsimd.dma_start(out=xt0[:, 5:8, :], in_=XT[:, 5:8, 0:CHUNK])
            nc.sync.dma_start(out=wt_sb[:, :, 128:256], in_=WT[:, :, 128:256])
            nc.gpsimd.dma_start(out=wt_sb[:, :, 256:512], in_=WT[:, :, 256:512])
            nc.sync.dma_start(out=cos_sb[:], in_=COS[:])
            nc.sync.dma_start(out=sin_sb[:], in_=SIN[:])
            nc.sync.dma_start(out=indt_sb[:], in_=INDT[:])
            nc.sync.dma_start(out=i2_sb[:], in_=I2[:])
            nc.sync.dma_start(out=lng_sb[:].bitcast(F32R), in_=LNG[:].bitcast(F32R))
            nc.sync.dma_start(out=wt_sb[:, :, 512:768], in_=WT[:, :, 512:768])
            nc.sync.dma_start(out=tri_sb[:], in_=TRI[:])
            nc.sync.dma_start(out=wot_sb[:], in_=WOT[:])
            emit_xt(1)

            # ones column 64 of each v block for the denominator trick (data
            # cols are overwritten by the V projection); gpsimd keeps it off
            # the DVE queue
            nc.gpsimd.memset(dum_sb[:], 1.0)
            nc.gpsimd.memset(v_sb[:], 1.0)
            # pre-load the combined ln+exp act table so the table-load pass
            # (greedy first-fit per function) never has to swap tables
            nc.scalar.add_instruction(mybir.InstLoadActFuncSet(
                name=nc.get_next_instruction_name(),
                act_func_set_id=6, engine=mybir.EngineType.Activation))
            dln = tmp.tile([1, 64], F32, tag="t", name="dln")
            nc.scalar.activation(dln[:], dum_sb[:], AF.Ln)

            def projqk_units(c, qk, units, tail):
                c0 = c * CHUNK
                qoff = 256 * qk
                dst = q_sb if qk == 0 else k_sb
                st = {}

                def mk_mm(which, di, qo):
                    def u(which=which, di=di, qo=qo):
                        if di == 0:
                            st[which] = ps.tile(
                                [128, CHUNK], F32, tag="pp",
                                name=f"p{which}_{qk}_{c}")
                        nc.tensor.matmul(
                            st[which],
                            lhsT=wt_sb[:, di, qo:qo + 128],
                            rhs=xts[c][:, di, :],
                            start=(di == 0), stop=(di == 7),
                        )
                    return u

                for di in range(8):
                    units.append(mk_mm("A", di, qoff))

                def uA():
                    pAc = tmp.tile([128, CHUNK], BF16, tag="t",
                                   name=f"pAc_{qk}_{c}")
                    nc.scalar.copy(pAc[:], st["A"][:])
                    sqA = tmp.tile([128, CHUNK], BF16, tag="t",
                                   name=f"sqA_{qk}_{c}")
                    nc.vector.tensor_mul(sqA[:], pAc[:], pAc[:])
                    st["Ac"], st["sqA"] = pAc, sqA
                units.append(uA)

                for di in range(8):
                    units.append(mk_mm("B", di, qoff + 128))

                def uB():
                    pBc = tmp.tile([128, CHUNK], BF16, tag="t",
                                   name=f"pBc_{qk}_{c}")
                    nc.scalar.copy(pBc[:], st["B"][:])
                    sqB = tmp.tile([128, CHUNK], BF16, tag="t",
                                   name=f"sqB_{qk}_{c}")
                    nc.vector.tensor_mul(sqB[:], pBc[:], pBc[:])
                    ssum = tmp.tile([128, CHUNK], BF16, tag="t",
                                    name=f"ssum_{qk}_{c}")
                    nc.vector.tensor_add(ssum[:], st["sqA"][:], sqB[:])
                    st["Bc"], st["ssum"] = pBc, ssum
                units.append(uB)

                cs = cos_sb[:, c0:c0 + CHUNK]
                sn = sin_sb[:, c0:c0 + CHUNK]

                if qk == 0:
                    # per-(head, position) g/||q||: n2 -> exp(-.5 ln + ln g)
                    def uN():
                        n2 = ps.tile([4, CHUNK], F32, tag="mm", name=f"n2_{c}")
                        nc.tensor.matmul(n2, lhsT=indt_sb[:], rhs=st["ssum"][:],
                                         start=True, stop=True)
                        lnq = tmp.tile([4, CHUNK], F32, tag="t", name=f"lnq_{c}")
                        nc.scalar.activation(lnq[:], n2[:], AF.Ln)
                        rbq = tmp.tile([4, CHUNK], BF16, tag="t", name=f"rbq_{c}")
                        nc.scalar.activation(rbq[:], lnq[:], AF.Exp,
                                             bias=lng_sb[:], scale=-0.5)
                        st["rbq"] = rbq
                    tail.append(uN)

                    def uBC():
                        rbp = ps.tile([128, CHUNK], F32, tag="mm", name=f"rbp_{c}")
                        nc.tensor.matmul(rbp, lhsT=i2_sb[:], rhs=st["rbq"][:],
                                         start=True, stop=True)
                        rb = tmp.tile([128, CHUNK], BF16, tag="t", name=f"rb_{c}")
                        nc.scalar.copy(rb[:], rbp[:])
                        rbc = tmp.tile([128, CHUNK], BF16, tag="t", name=f"rbc_{c}")
                        nc.vector.tensor_mul(rbc[:], rb[:], cs)
                        rbs = tmp.tile([128, CHUNK], BF16, tag="t", name=f"rbs_{c}")
                        nc.vector.tensor_mul(rbs[:], rb[:], sn)
                        st["rbc"], st["rbs"] = rbc, rbs
                    tail.append(uBC)
                else:
                    # per-key 1/||k||, transposed to [key, head] for use as
                    # the exp() scale operand
                    def uKN():
                        kn = ps.tile([128, 16], F32, tag="mm", name=f"kn_{c}")
                        for bb in range(4):
                            nc.tensor.matmul(
                                kn[:, 4 * bb:4 * bb + 4],
                                lhsT=st["ssum"][:, bb * 128:bb * 128 + 128],
                                rhs=indt_sb[:], start=True, stop=True,
                            )
                        lnk = tmp.tile([128, 16], F32, tag="kt", name=f"lnk_{c}")
                        nc.scalar.activation(lnk[:], kn[:], AF.Ln)
                        nc.scalar.activation(kre_sb[:, 16 * c:16 * c + 16],
                                             lnk[:], AF.Exp, scale=-0.5)
                    tail.append(uKN)

                def uProd():
                    pc = st["rbc"][:] if qk == 0 else cs
                    pss = st["rbs"][:] if qk == 0 else sn
                    for nm, src, mulby in (("tac", "Ac", pc), ("tas", "Ac", pss),
                                           ("tbc", "Bc", pc), ("tbs", "Bc", pss)):
                        t = tmp.tile([128, CHUNK], BF16, tag="t",
                                     name=f"{nm}_{qk}_{c}")
                        nc.vector.tensor_mul(t[:], st[src][:], mulby)
                        st[nm] = t
                tail.append(uProd)

                def uComb():
                    for h in range(HPC):
                        po = (h % 2) * 64
                        ti = h // 2
                        hs = 32 * h
                        nc.vector.tensor_sub(
                            dst[po:po + 32, ti, c0:c0 + CHUNK],
                            st["tac"][hs:hs + 32, :], st["tbs"][hs:hs + 32, :])
                        nc.vector.tensor_add(
                            dst[po + 32:po + 64, ti, c0:c0 + CHUNK],
                            st["tas"][hs:hs + 32, :], st["tbc"][hs:hs + 32, :])
                tail.append(uComb)

            def projv_units(c, units):
                for bb in range(4):
                    nb = 4 * c + bb
                    st = {}

                    def u1(bb=bb, nb=nb, st=st):
                        st["v"] = ps.tile([128, HPC, 64], F32, tag="mm",
                                          name=f"vps_{nb}")
                        for di in range(4):
                            nc.tensor.matmul(
                                st["v"],
                                lhsT=xts[c][:, di, bb * 128:bb * 128 + 128],
                                rhs=wt_sb[:, di, 512:768],
                                start=(di == 0), stop=False,
                            )

                    def u2(bb=bb, nb=nb, st=st):
                        for di in range(4, 8):
                            nc.tensor.matmul(
                                st["v"],
                                lhsT=xts[c][:, di, bb * 128:bb * 128 + 128],
                                rhs=wt_sb[:, di, 512:768],
                                start=False, stop=(di == 7),
                            )
                        nc.vector.tensor_copy(v_sb[:, nb, :, 0:64], st["v"][:])
                    units.append(u1)
                    units.append(u2)

            def outproj_units(c, units):
                for bb in range(4):
                    nb = 4 * c + bb
                    for oc in range(2):
                        def u(nb=nb, oc=oc):
                            wo = ps.tile([128, CHUNK], F32, tag="mm",
                                         name=f"wo_{nb}_{oc}")
                            for ti in range(2):
                                nc.tensor.matmul(
                                    wo,
                                    lhsT=ot_sb[:, ti, nb * 128:nb * 128 + 128],
                                    rhs=wot_sb[:, ti, oc * CHUNK:oc * CHUNK + CHUNK],
                                    start=(ti == 0), stop=(ti == 1),
                                )
                            ob = ocp.tile([128, CHUNK], F32, tag="ob",
                                          name=f"ob_{nb}_{oc}")
                            nc.vector.tensor_copy(ob[:], wo[:])
                            nc.sync.dma_start(
                                out=OUT[nb * 128:nb * 128 + 128,
                                        oc * CHUNK:oc * CHUNK + CHUNK],
                                in_=ob[:])
                        units.append(u)

            def mk_fill(units):
                state = {"i": 0}

                def fill(n):
                    while n > 0 and state["i"] < len(units):
                        units[state["i"]]()
                        state["i"] += 1
                        n -= 1
                return fill

            def emit_attn_head(c, h, fill, ot_piece_cb=None):
                c0 = c * CHUNK
                njb = 4 * (c + 1)
                po = (h % 2) * 64
                ti = h // 2
                av = ps.tile([65, CHUNK], F32, tag="av", name=f"av_{h}_{c}")

                def issue_sc(jb):
                    bb = jb - 4 * c
                    lo = 128 * bb if bb > 0 else 0
                    sc = ps.tile([128, CHUNK], F32, tag="sc",
                                 name=f"sc_{h}_{c}_{jb}")
                    nc.tensor.matmul(
                        sc[:, lo:],
                        lhsT=k_sb[po:po + 64, ti, jb * 128:jb * 128 + 128],
                        rhs=q_sb[po:po + 64, ti, c0 + lo:c0 + CHUNK],
                        start=True, stop=True,
                    )
                    return sc, lo

                # stagger: issue sc for jb+1 before draining jb so PE keeps
                # ahead of ACT's exp stream; fill PE bubbles with proj work
                cur = issue_sc(0)
                for jb in range(njb):
                    nxt = issue_sc(jb + 1) if jb + 1 < njb else None
                    sc, lo = cur
                    diag = jb >= 4 * c
                    ex = expp.tile([128, CHUNK], BF16, tag="ex",
                                   name=f"ex_{h}_{c}_{jb}")
                    nc.scalar.activation(
                        ex[:, lo:], sc[:, lo:], AF.Exp,
                        scale=kre_sb[:, 4 * jb + h:4 * jb + h + 1])
                    if diag:
                        # zero the strictly-upper triangle of the 128-col
                        # window at the causal boundary
                        nc.vector.tensor_mul(
                            ex[:, lo:lo + 128], ex[:, lo:lo + 128], tri_sb[:])
                    nc.tensor.matmul(
                        av[:, lo:], lhsT=v_sb[:, jb, h, :], rhs=ex[:, lo:],
                        start=(jb == 0), stop=(jb == njb - 1),
                        skip_group_check=True,
                    )
                    fill(1)
                    cur = nxt

                srec = bcp.tile([1, CHUNK], F32, tag="srec", name=f"srec_{h}_{c}")
                nc.vector.reciprocal(srec[:].bitcast(F32R), av[64:65, :])
                rb2 = bcp.tile([64, CHUNK], F32, tag="rb2", name=f"rb2_{h}_{c}")
                nc.gpsimd.partition_broadcast(rb2[:], srec[0:1, :])
                if ot_piece_cb is None:
                    nc.vector.tensor_mul(
                        ot_sb[po:po + 64, ti, c0:c0 + CHUNK],
                        av[0:64, :], rb2[:])
                else:
                    # last head of the last chunk: emit ot in 128-col pieces
                    # so the final output projection pipelines behind it
                    for p in range(4):
                        lo = 128 * p
                        nc.vector.tensor_mul(
                            ot_sb[po:po + 64, ti, c0 + lo:c0 + lo + 128],
                            av[0:64, lo:lo + 128], rb2[:, lo:lo + 128])
                        ot_piece_cb(p)

            # warm-up: chunk 0 projections run back-to-back (no attention
            # yet to interleave with); bulk matmuls first so PE has a long
            # runway while the DVE/ACT rope chain catches up
            bulk0, tail0 = [], []
            projqk_units(0, 0, bulk0, tail0)
            projqk_units(0, 1, bulk0, tail0)
            projv_units(0, bulk0)
            for u in bulk0 + tail0:
                u()

            # steady phases: attention for chunk c-1 interleaved, at matmul
            # granularity, with chunk c's projections and earlier chunks'
            # output projections so PE never starves while ACT streams exps.
            # outproj(1) and outproj(2) are held back as fill for the final
            # (projection-free, exp-bound) phase.
            for c in range(1, NCH):
                units, tail = [], []
                if c + 1 < NCH:
                    units.append(lambda c=c: emit_xt(c + 1))
                if c == 2:
                    outproj_units(0, units)
                projqk_units(c, 0, units, tail)
                projqk_units(c, 1, units, tail)
                projv_units(c, units)
                units += tail
                fill = mk_fill(units)
                per_block = 2 if c == 1 else 1
                for h in range(HPC):
                    fill(2)
                    emit_attn_head(c - 1, h, lambda n: fill(n * per_block))
                    fill(4)
                fill(len(units))

            units = []
            outproj_units(NCH - 3, units)
            outproj_units(NCH - 2, units)
            unitsF = []
            outproj_units(NCH - 1, unitsF)
            fillF = mk_fill(unitsF)
            fill = mk_fill(units)
            for h in range(HPC):
                fill(2)
                if h < HPC - 1:
                    emit_attn_head(NCH - 1, h, fill)
                else:
                    emit_attn_head(NCH - 1, h, fill,
                                   ot_piece_cb=lambda p: fillF(2))
                fill(4)
            fill(len(units))
            fillF(len(unitsF))
    return nc


_NC = None


def _get_nc():
    global _NC
    if _NC is None:
        _NC = _build_nc()
        _NC.finalize()
    return _NC


def _shared_tables(token_positions):
    freqs = np.arange(0, DK, 2, dtype=np.float64)
    inv_theta = THETA ** (-freqs / DK)                      # [32]
    pos = token_positions.astype(np.float64)
    ang = inv_theta[:, None] * pos[None, :]                 # [32, SEQ]
    cos_t = np.ascontiguousarray(
        np.tile(np.cos(ang), (4, 1))).astype(ml_dtypes.bfloat16)
    sin_t = np.ascontiguousarray(
        np.tile(np.sin(ang), (4, 1))).astype(ml_dtypes.bfloat16)

    indt = np.zeros((128, 4), dtype=np.float32)
    for j in range(4):
        indt[32 * j:32 * j + 32, j] = 1.0
    i2 = np.ascontiguousarray(indt.T).astype(ml_dtypes.bfloat16)

    p_i = np.arange(128)[:, None]
    t_i = np.arange(128)[None, :]
    tri = (p_i <= t_i).astype(ml_dtypes.bfloat16)
    return cos_t, sin_t, indt.astype(ml_dtypes.bfloat16), i2, tri


def _core_inputs(c, x, W_QKV, W_O, qk_scale, shared):
    cos_t, sin_t, indt, i2, tri = shared
    b = c // 4
    a = c % 4
    heads = [4 * a + i for i in range(HPC)]

    qA = [64 * h + 2 * t for h in heads for t in range(32)]
    qB = [64 * h + 2 * t + 1 for h in heads for t in range(32)]
    kA = [1024 + r for r in qA]
    kB = [1024 + r for r in qB]
    vr = [2048 + 64 * h + j for h in heads for j in range(DK)]
    rows = qA + qB + kA + kB + vr
    wt = np.ascontiguousarray(
        W_QKV[rows, :].T.reshape(8, 128, 768).transpose(1, 0, 2)
    ).astype(ml_dtypes.bfloat16)

    vcols = [64 * h + j for h in heads for j in range(DK)]
    wot = np.ascontiguousarray(
        W_O[:, vcols].T.reshape(2, 128, D).transpose(1, 0, 2)
    ).astype(ml_dtypes.bfloat16)

    xt = np.ascontiguousarray(
        x[b].T.reshape(8, 128, SEQ).transpose(1, 0, 2)
    ).astype(ml_dtypes.bfloat16)

    lng = np.log(qk_scale[heads].astype(np.float64)).astype(
        np.float32).reshape(4, 1)

    return {
        "XT": xt, "WT": wt, "COS": cos_t, "SIN": sin_t, "WOT": wot,
        "INDT": indt, "I2": i2, "LNG": lng, "TRI": tri,
    }


def _run(inputs, trace=False):
    x = np.asarray(inputs["x"], dtype=np.float32)
    token_positions = np.asarray(inputs["token_positions"])
    W_QKV = np.asarray(inputs["W_QKV"], dtype=np.float32)
    W_O = np.asarray(inputs["W_O"], dtype=np.float32)
    qk_scale = np.asarray(inputs["qk_scale"], dtype=np.float32)

    shared = _shared_tables(token_positions)
    nc = _get_nc()
    in_maps = [_core_inputs(c, x, W_QKV, W_O, qk_scale, shared)
               for c in range(NCORES)]
    core_ids = list(range(NCORES))
    kw = {}
    if trace:
        kw = dict(trace=True, trace_cores=core_ids)
    res = run_bass_kernel_spmd(nc, in_maps, core_ids, **kw)
    parts = [np.asarray(r["OUT"], dtype=np.float32) for r in res.results]
    out = np.stack([
        parts[0] + parts[1] + parts[2] + parts[3],
        parts[4] + parts[5] + parts[6] + parts[7],
    ]).astype(np.float32)
    return out, getattr(res, "exec_time_ns", None)


def kernel(**inputs):
    return _run(inputs, trace=False)[0]


def estimate_time_ns():
    from concourse.timeline_sim import TimelineSim
    ts = TimelineSim(_get_nc(), trace=False, no_exec=True)
    return ts.simulate()


def kernel_timed(**inputs):
    out, _ = _run(inputs, trace=False)
    return out, estimate_time_ns()


# revision 9
# speedup vs baseline: 1.3292x; 1.0058x over previous
import sys

sys.path.insert(0, "/opt/trn_rl_repo")

import ml_dtypes
import numpy as np

import concourse.bass as bass
import concourse.tile as tile
from concourse import bacc, mybir
from concourse.bass_utils import run_bass_kernel_spmd

F32 = mybir.dt.float32
F32R = mybir.dt.float32r
BF16 = mybir.dt.bfloat16
AF = mybir.ActivationFunctionType

BATCH = 2
SEQ = 2048
D = 1024
NHEADS = 16
DK = 64
HPC = 4          # heads per core
NCORES = 8
THETA = 10000.0
CHUNK = 512
NCH = SEQ // CHUNK   # 4 chunks of queries
NBLK = SEQ // 128    # 16 key blocks


def _build_nc():
    nc = bacc.Bacc("TRN2", target_bir_lowering=False)
    XT = nc.declare_dram_parameter("XT", [128, 8, SEQ], BF16, isOutput=False)
    WT = nc.declare_dram_parameter("WT", [128, 8, 768], BF16, isOutput=False)
    COS = nc.declare_dram_parameter("COS", [128, SEQ], BF16, isOutput=False)
    SIN = nc.declare_dram_parameter("SIN", [128, SEQ], BF16, isOutput=False)
    WOT = nc.declare_dram_parameter("WOT", [128, 2, D], BF16, isOutput=False)
    INDT = nc.declare_dram_parameter("INDT", [128, 4], BF16, isOutput=False)
    I2 = nc.declare_dram_parameter("I2", [4, 128], BF16, isOutput=False)
    LNG = nc.declare_dram_parameter("LNG", [4, 1], F32, isOutput=False)
    TRI = nc.declare_dram_parameter("TRI", [128, 128], BF16, isOutput=False)
    OUT = nc.declare_dram_parameter("OUT", [SEQ, D], F32, isOutput=True)

    with tile.TileContext(nc) as tc:
        with (
            nc.allow_low_precision(reason="bf16 matmuls validated at 1e-2 rel err"),
            tc.tile_pool(name="cst", bufs=1) as cst,
            tc.tile_pool(name="xtp", bufs=2) as xtp,
            tc.tile_pool(name="tmp", bufs=10) as tmp,
            tc.tile_pool(name="expp", bufs=3) as expp,
            tc.tile_pool(name="bcp", bufs=2) as bcp,
            tc.tile_pool(name="ocp", bufs=2) as ocp,
            tc.tile_pool(name="ps", bufs=2, space="PSUM") as ps,
        ):
            wt_sb = cst.tile([128, 8, 768], BF16, tag="wt")
            cos_sb = cst.tile([128, SEQ], BF16, tag="cos")
            sin_sb = cst.tile([128, SEQ], BF16, tag="sin")
            wot_sb = cst.tile([128, 2, D], BF16, tag="wot")
            indt_sb = cst.tile([128, 4], BF16, tag="indt")
            i2_sb = cst.tile([4, 128], BF16, tag="i2")
            lng_sb = cst.tile([4, 1], F32, tag="lng")
            tri_sb = cst.tile([128, 128], BF16, tag="tri")
            q_sb = cst.tile([128, 2, SEQ], BF16, tag="q")
            k_sb = cst.tile([128, 2, SEQ], BF16, tag="k")
            v_sb = cst.tile([128, NBLK, HPC, 65], BF16, tag="v")
            ot_sb = cst.tile([128, 2, SEQ], BF16, tag="ot")
            kre_sb = cst.tile([128, NBLK * HPC], F32, tag="kre")
            dum_sb = cst.tile([1, 64], F32, tag="dum")

            xts = {}

            def emit_xt(c):
                c0 = c * CHUNK
                xt_t = xtp.tile([128, 8, CHUNK], BF16, tag="xt", name=f"xt_{c}")
                nc.sync.dma_start(out=xt_t[:, 0:4, :], in_=XT[:, 0:4, c0:c0 + CHUNK])
                nc.sync.dma_start(out=xt_t[:, 4:8, :], in_=XT[:, 4:8, c0:c0 + CHUNK])
                xts[c] = xt_t

            # startup: first proj needs xt(0) di-slices + Q weight columns;
            # spread the first pieces across engine DGE queues (SP / Pool /
            # ACT / DVE run their DMAs concurrently) so the first matmul can
            # start as early as possible
            xt0 = xtp.tile([128, 8, CHUNK], BF16, tag="xt", name="xt_0")
            xts[0] = xt0
            nc.sync.dma_start(out=wt_sb[:, 0:1, 0:128], in_=WT[:, 0:1, 0:128])
            nc.gpsimd.dma_start(out=xt0[:, 0:2, :], in_=XT[:, 0:2, 0:CHUNK])
            nc.scalar.dma_start(out=xt0[:, 2:5, :], in_=XT[:, 2:5, 0:CHUNK])
            nc.sync.dma_start(out=wt_sb[:, 1:8, 0:128], in_=WT[:, 1:8, 0:128])
            nc.gpsimd.dma_start(out=xt0[:, 5:8, :], in_=XT[:, 5:8, 0:CHUNK])
            nc.sync.dma_start(out=wt_sb[:, :, 128:256], in_=WT[:, :, 128:256])
            nc.gpsimd.dma_start(out=wt_sb[:, :, 256:512], in_=WT[:, :, 256:512])
            nc.sync.dma_start(out=cos_sb[:], in_=COS[:])
            nc.sync.dma_start(out=sin_sb[:], in_=SIN[:])
            nc.sync.dma_start(out=indt_sb[:], in_=INDT[:])
            nc.sync.dma_start(out=i2_sb[:], in_=I2[:])
            nc.sync.dma_start(out=lng_sb[:].bitcast(F32R), in_=LNG[:].bitcast(F32R))
            nc.sync.dma_start(out=wt_sb[:, :, 512:768], in_=WT[:, :, 512:768])
            nc.sync.dma_start(out=tri_sb[:], in_=TRI[:])
            nc.sync.dma_start(out=wot_sb[:], in_=WOT[:])
            emit_xt(1)

            # ones column 64 of each v block for the denominator trick (data
            # cols are overwritten by the V projection); gpsimd keeps it off
            # the DVE queue
            nc.gpsimd.memset(dum_sb[:], 1.0)
            nc.gpsimd.memset(v_sb[:], 1.0)
            # pre-load the combined ln+exp act table so the table-load pass
            # (greedy first-fit per function) never has to swap tables
            nc.scalar.add_instruction(mybir.InstLoadActFuncSet(
                name=nc.get_next_instruction_name(),
                act_func_set_id=6, engine=mybir.EngineType.Activation))
            dln = tmp.tile([1, 64], F32, tag="t", name="dln")
            nc.scalar.activation(dln[:], dum_sb[:], AF.Ln)

            def projqk_units(c, qk, units, tail):
                c0 = c * CHUNK
                qoff = 256 * qk
                dst = q_sb if qk == 0 else k_sb
                st = {}

                def mk_mm(which, di, qo):
                    def u(which=which, di=di, qo=qo):
                        if di == 0:
                            st[which] = ps.tile(
                                [128, CHUNK], F32, tag="pp",
                                name=f"p{which}_{qk}_{c}")
                        nc.tensor.matmul(
                            st[which],
                            lhsT=wt_sb[:, di, qo:qo + 128],
                            rhs=xts[c][:, di, :],
                            start=(di == 0), stop=(di == 7),
                        )
                    return u

                for di in range(8):
                    units.append(mk_mm("A", di, qoff))

                def uA():
                    pAc = tmp.tile([128, CHUNK], BF16, tag="t",
                                   name=f"pAc_{qk}_{c}")
                    nc.scalar.copy(pAc[:], st["A"][:])
                    sqA = tmp.tile([128, CHUNK], BF16, tag="t",
                                   name=f"sqA_{qk}_{c}")
                    nc.vector.tensor_mul(sqA[:], pAc[:], pAc[:])
                    st["Ac"], st["sqA"] = pAc, sqA
                units.append(uA)

                for di in range(8):
                    units.append(mk_mm("B", di, qoff + 128))

                def uB():
                    pBc = tmp.tile([128, CHUNK], BF16, tag="t",
                                   name=f"pBc_{qk}_{c}")
                    nc.scalar.copy(pBc[:], st["B"][:])
                    sqB = tmp.tile([128, CHUNK], BF16, tag="t",
                                   name=f"sqB_{qk}_{c}")
                    nc.vector.tensor_mul(sqB[:], pBc[:], pBc[:])
                    ssum = tmp.tile([128, CHUNK], BF16, tag="t",
                                    name=f"ssum_{qk}_{c}")
                    nc.vector.tensor_add(ssum[:], st["sqA"][:], sqB[:])
                    st["Bc"], st["ssum"] = pBc, ssum
                units.append(uB)

                cs = cos_sb[:, c0:c0 + CHUNK]
                sn = sin_sb[:, c0:c0 + CHUNK]

                if qk == 0:
                    # per-(head, position) g/||q||: n2 -> exp(-.5 ln + ln g)
                    def uN():
                        n2 = ps.tile([4, CHUNK], F32, tag="mm", name=f"n2_{c}")
                        nc.tensor.matmul(n2, lhsT=indt_sb[:], rhs=st["ssum"][:],
                                         start=True, stop=True)
                        lnq = tmp.tile([4, CHUNK], F32, tag="t", name=f"lnq_{c}")
                        nc.scalar.activation(lnq[:], n2[:], AF.Ln)
                        rbq = tmp.tile([4, CHUNK], BF16, tag="t", name=f"rbq_{c}")
                        nc.scalar.activation(rbq[:], lnq[:], AF.Exp,
                                             bias=lng_sb[:], scale=-0.5)
                        st["rbq"] = rbq
                    tail.append(uN)

                    def uBC():
                        rbp = ps.tile([128, CHUNK], F32, tag="mm", name=f"rbp_{c}")
                        nc.tensor.matmul(rbp, lhsT=i2_sb[:], rhs=st["rbq"][:],
                                         start=True, stop=True)
                        rb = tmp.tile([128, CHUNK], BF16, tag="t", name=f"rb_{c}")
                        nc.scalar.copy(rb[:], rbp[:])
                        rbc = tmp.tile([128, CHUNK], BF16, tag="t", name=f"rbc_{c}")
                        nc.vector.tensor_mul(rbc[:], rb[:], cs)
                        rbs = tmp.tile([128, CHUNK], BF16, tag="t", name=f"rbs_{c}")
                        nc.vector.tensor_mul(rbs[:], rb[:], sn)
                        st["rbc"], st["rbs"] = rbc, rbs
                    tail.append(uBC)
                else:
                    # per-key 1/||k||, transposed to [key, head] for use as
                    # the exp() scale operand
                    def uKN():
                        kn = ps.tile([128, 16], F32, tag="mm", name=f"kn_{c}")
                        for bb in range(4):
                            nc.tensor.matmul(
                                kn[:, 4 * bb:4 * bb + 4],
                                lhsT=st["ssum"][:, bb * 128:bb * 128 + 128],
                                rhs=indt_sb[:], start=True, stop=True,
                            )
                        lnk = tmp.tile([128, 16], F32, tag="kt", name=f"lnk_{c}")
                        nc.scalar.activation(lnk[:], kn[:], AF.Ln)
                        nc.scalar.activation(kre_sb[:, 16 * c:16 * c + 16],
                                             lnk[:], AF.Exp, scale=-0.5)
                    tail.append(uKN)

                def uProd():
                    pc = st["rbc"][:] if qk == 0 else cs
                    pss = st["rbs"][:] if qk == 0 else sn
                    for nm, src, mulby in (("tac", "Ac", pc), ("tas", "Ac", pss),
                                           ("tbc", "Bc", pc), ("tbs", "Bc", pss)):
                        t = tmp.tile([128, CHUNK], BF16, tag="t",
                                     name=f"{nm}_{qk}_{c}")
                        nc.vector.tensor_mul(t[:], st[src][:], mulby)
                        st[nm] = t
                tail.append(uProd)

                def uComb():
                    for h in range(HPC):
                        po = (h % 2) * 64
                        ti = h // 2
                        hs = 32 * h
                        nc.vector.tensor_sub(
                            dst[po:po + 32, ti, c0:c0 + CHUNK],
                            st["tac"][hs:hs + 32, :], st["tbs"][hs:hs + 32, :])
                        nc.vector.tensor_add(
                            dst[po + 32:po + 64, ti, c0:c0 + CHUNK],
                            st["tas"][hs:hs + 32, :], st["tbc"][hs:hs + 32, :])
                tail.append(uComb)

            def projv_units(c, units):
                for bb in range(4):
                    nb = 4 * c + bb
                    st = {}

                    def u1(bb=bb, nb=nb, st=st):
                        st["v"] = ps.tile([128, HPC, 64], F32, tag="mm",
                                          name=f"vps_{nb}")
                        for di in range(4):
                            nc.tensor.matmul(
                                st["v"],
                                lhsT=xts[c][:, di, bb * 128:bb * 128 + 128],
                                rhs=wt_sb[:, di, 512:768],
                                start=(di == 0), stop=False,
                            )

                    def u2(bb=bb, nb=nb, st=st):
                        for di in range(4, 8):
                            nc.tensor.matmul(
                                st["v"],
                                lhsT=xts[c][:, di, bb * 128:bb * 128 + 128],
                                rhs=wt_sb[:, di, 512:768],
                                start=False, stop=(di == 7),
                            )
                        nc.vector.tensor_copy(v_sb[:, nb, :, 0:64], st["v"][:])
                    units.append(u1)
                    units.append(u2)

            def outproj_units(c, units, act_copy=False):
                for bb in range(4):
                    nb = 4 * c + bb
                    for oc in range(2):
                        def u(nb=nb, oc=oc):
                            wo = ps.tile([128, CHUNK], F32, tag="mm",
                                         name=f"wo_{nb}_{oc}")
                            for ti in range(2):
                                nc.tensor.matmul(
                                    wo,
                                    lhsT=ot_sb[:, ti, nb * 128:nb * 128 + 128],
                                    rhs=wot_sb[:, ti, oc * CHUNK:oc * CHUNK + CHUNK],
                                    start=(ti == 0), stop=(ti == 1),
                                )
                            ob = ocp.tile([128, CHUNK], F32, tag="ob",
                                          name=f"ob_{nb}_{oc}")
                            if act_copy:
                                nc.scalar.copy(ob[:], wo[:])
                            else:
                                nc.vector.tensor_copy(ob[:], wo[:])
                            nc.sync.dma_start(
                                out=OUT[nb * 128:nb * 128 + 128,
                                        oc * CHUNK:oc * CHUNK + CHUNK],
                                in_=ob[:])
                        units.append(u)

            def mk_fill(units, rate=1.0):
                state = {"i": 0, "cr": 0.0}

                def fill(n):
                    state["cr"] += n * rate
                    while state["cr"] >= 1.0 and state["i"] < len(units):
                        units[state["i"]]()
                        state["i"] += 1
                        state["cr"] -= 1.0
                return fill

            def emit_attn_head(c, h, fill, ot_piece_cb=None):
                c0 = c * CHUNK
                njb = 4 * (c + 1)
                po = (h % 2) * 64
                ti = h // 2
                av = ps.tile([65, CHUNK], F32, tag="av", name=f"av_{h}_{c}")

                def issue_sc(jb):
                    bb = jb - 4 * c
                    lo = 128 * bb if bb > 0 else 0
                    sc = ps.tile([128, CHUNK], F32, tag="sc",
                                 name=f"sc_{h}_{c}_{jb}")
                    nc.tensor.matmul(
                        sc[:, lo:],
                        lhsT=k_sb[po:po + 64, ti, jb * 128:jb * 128 + 128],
                        rhs=q_sb[po:po + 64, ti, c0 + lo:c0 + CHUNK],
                        start=True, stop=True,
                    )
                    return sc, lo

                # stagger: issue sc for jb+1 before draining jb so PE keeps
                # ahead of ACT's exp stream; fill PE bubbles with proj work
                cur = issue_sc(0)
                for jb in range(njb):
                    nxt = issue_sc(jb + 1) if jb + 1 < njb else None
                    sc, lo = cur
                    diag = jb >= 4 * c
                    ex = expp.tile([128, CHUNK], BF16, tag="ex",
                                   name=f"ex_{h}_{c}_{jb}")
                    nc.scalar.activation(
                        ex[:, lo:], sc[:, lo:], AF.Exp,
                        scale=kre_sb[:, 4 * jb + h:4 * jb + h + 1])
                    if diag:
                        # zero the strictly-upper triangle of the 128-col
                        # window at the causal boundary
                        nc.vector.tensor_mul(
                            ex[:, lo:lo + 128], ex[:, lo:lo + 128], tri_sb[:])
                    nc.tensor.matmul(
                        av[:, lo:], lhsT=v_sb[:, jb, h, :], rhs=ex[:, lo:],
                        start=(jb == 0), stop=(jb == njb - 1),
                        skip_group_check=True,
                    )
                    fill(1)
                    cur = nxt

                srec = bcp.tile([1, CHUNK], F32, tag="srec", name=f"srec_{h}_{c}")
                nc.vector.reciprocal(srec[:].bitcast(F32R), av[64:65, :])
                rb2 = bcp.tile([64, CHUNK], F32, tag="rb2", name=f"rb2_{h}_{c}")
                nc.gpsimd.partition_broadcast(rb2[:], srec[0:1, :])
                if ot_piece_cb is None:
                    nc.vector.tensor_mul(
                        ot_sb[po:po + 64, ti, c0:c0 + CHUNK],
                        av[0:64, :], rb2[:])
                else:
                    # last head of the last chunk: emit ot in 128-col pieces
                    # so the final output projection pipelines behind it
                    for p in range(4):
                        lo = 128 * p
                        nc.vector.tensor_mul(
                            ot_sb[po:po + 64, ti, c0 + lo:c0 + lo + 128],
                            av[0:64, lo:lo + 128], rb2[:, lo:lo + 128])
                        ot_piece_cb(p)

            # warm-up: chunk 0 projections run back-to-back (no attention
            # yet to interleave with); bulk matmuls first so PE has a long
            # runway while the DVE/ACT rope chain catches up
            bulk0, tail0 = [], []
            projqk_units(0, 0, bulk0, tail0)
            projqk_units(0, 1, bulk0, tail0)
            projv_units(0, tail0)
            for u in bulk0 + tail0:
                u()

            # steady phases: attention for chunk c-1 interleaved, at matmul
            # granularity, with chunk c's projections and earlier chunks'
            # output projections so PE never starves while ACT streams exps.
            # outproj(1) and outproj(2) are held back as fill for the final
            # (projection-free, exp-bound) phase.
            for c in range(1, NCH):
                units, tail = [], []
                if c + 1 < NCH:
                    units.append(lambda c=c: emit_xt(c + 1))
                if c == 2:
                    outproj_units(0, units)
                projqk_units(c, 0, units, tail)
                projqk_units(c, 1, units, tail)
                projv_units(c, units)
                units += tail
                blocks = 4 * c * HPC
                credits = blocks + 6 * HPC
                rate = min(1.6, len(units) / credits * 1.25)
                fill = mk_fill(units, rate)
                for h in range(HPC):
                    fill(2 / rate)
                    emit_attn_head(c - 1, h, fill)
                    fill(4)
                fill(10**6)

            units = []
            outproj_units(NCH - 3, units)
            outproj_units(NCH - 2, units)
            unitsF = []
            outproj_units(NCH - 1, unitsF, act_copy=True)
            fillF = mk_fill(unitsF)
            blocks = 4 * NCH * HPC
            rate = len(units) / (blocks + 6 * HPC)
            fill = mk_fill(units, rate)
            for h in range(HPC):
                fill(2 / rate)
                if h < HPC - 1:
                    emit_attn_head(NCH - 1, h, fill)
                else:
                    fill(10**6)
                    emit_attn_head(NCH - 1, h, fill,
                                   ot_piece_cb=lambda p: fillF(2))
                fill(4)
            fill(10**6)
            fillF(10**6)
    return nc


_NC = None


def _get_nc():
    global _NC
    if _NC is None:
        _NC = _build_nc()
        _NC.finalize()
    return _NC


def _shared_tables(token_positions):
    freqs = np.arange(0, DK, 2, dtype=np.float64)
    inv_theta = THETA ** (-freqs / DK)                      # [32]
    pos = token_positions.astype(np.float64)
    ang = inv_theta[:, None] * pos[None, :]                 # [32, SEQ]
    cos_t = np.ascontiguousarray(
        np.tile(np.cos(ang), (4, 1))).astype(ml_dtypes.bfloat16)
    sin_t = np.ascontiguousarray(
        np.tile(np.sin(ang), (4, 1))).astype(ml_dtypes.bfloat16)

    indt = np.zeros((128, 4), dtype=np.float32)
    for j in range(4):
        indt[32 * j:32 * j + 32, j] = 1.0
    i2 = np.ascontiguousarray(indt.T).astype(ml_dtypes.bfloat16)

    p_i = np.arange(128)[:, None]
    t_i = np.arange(128)[None, :]
    tri = (p_i <= t_i).astype(ml_dtypes.bfloat16)
    return cos_t, sin_t, indt.astype(ml_dtypes.bfloat16), i2, tri


def _core_inputs(c, x, W_QKV, W_O, qk_scale, shared):
    cos_t, sin_t, indt, i2, tri = shared
    b = c // 4
    a = c % 4
    heads = [4 * a + i for i in range(HPC)]

    qA = [64 * h + 2 * t for h in heads for t in range(32)]
    qB = [64 * h + 2 * t + 1 for h in heads for t in range(32)]
    kA = [1024 + r for r in qA]
    kB = [1024 + r for r in qB]
    vr = [2048 + 64 * h + j for h in heads for j in range(DK)]
    rows = qA + qB + kA + kB + vr
    wt = np.ascontiguousarray(
        W_QKV[rows, :].T.reshape(8, 128, 768).transpose(1, 0, 2)
    ).astype(ml_dtypes.bfloat16)

    vcols = [64 * h + j for h in heads for j in range(DK)]
    wot = np.ascontiguousarray(
        W_O[:, vcols].T.reshape(2, 128, D).transpose(1, 0, 2)
    ).astype(ml_dtypes.bfloat16)

    xt = np.ascontiguousarray(
        x[b].T.reshape(8, 128, SEQ).transpose(1, 0, 2)
    ).astype(ml_dtypes.bfloat16)

    lng = np.log(qk_scale[heads].astype(np.float64)).astype(
        np.float32).reshape(4, 1)

    return {
        "XT": xt, "WT": wt, "COS": cos_t, "SIN": sin_t, "WOT": wot,
        "INDT": indt, "I2": i2, "LNG": lng, "TRI": tri,
    }


def _run(inputs, trace=False):
    x = np.asarray(inputs["x"], dtype=np.float32)
    token_positions = np.asarray(inputs["token_positions"])
    W_QKV = np.asarray(inputs["W_QKV"], dtype=np.float32)
    W_O = np.asarray(inputs["W_O"], dtype=np.float32)
    qk_scale = np.asarray(inputs["qk_scale"], dtype=np.float32)

    shared = _shared_tables(token_positions)
    nc = _get_nc()
    in_maps = [_core_inputs(c, x, W_QKV, W_O, qk_scale, shared)
               for c in range(NCORES)]
    core_ids = list(range(NCORES))
    kw = {}
    if trace:
        kw = dict(trace=True, trace_cores=core_ids)
    res = run_bass_kernel_spmd(nc, in_maps, core_ids, **kw)
    parts = [np.asarray(r["OUT"], dtype=np.float32) for r in res.results]
    out = np.stack([
        parts[0] + parts[1] + parts[2] + parts[3],
        parts[4] + parts[5] + parts[6] + parts[7],
    ]).astype(np.float32)
    return out, getattr(res, "exec_time_ns", None)


def kernel(**inputs):
    return _run(inputs, trace=False)[0]


def estimate_time_ns():
    from concourse.timeline_sim import TimelineSim
    ts = TimelineSim(_get_nc(), trace=False, no_exec=True)
    return ts.simulate()


def kernel_timed(**inputs):
    out, _ = _run(inputs, trace=False)
    return out, estimate_time_ns()
